# revision 39
# baseline (speedup 1.0000x reference)
"""Trainium2 Bass kernel for a dense pre-LN transformer block.

B=4, T=1024, C=1024, H=16 heads (head_size 64).

Distribution over the 8 NeuronCores (two SPMD launches, host-side
reduction between them):

  Launch A (attention, head-parallel): every core runs the identical
  program on all 4 batches but with its own pair of heads (weight
  slices are per-core input data). Each core computes LN1, its 2 heads'
  q/k/v + causal attention, and the partial Wo projection of those
  heads for the whole [B*T, C] output (written f32 from PSUM).
  NOTE the reference computes scores as k @ q^T (roles of q/k swapped
  vs standard attention) — handled by using k rows as the "queries".

  Host: x2 = x + sum_c projpart_c + bo.

  Launch B (FFN, row-parallel): core c runs LN2 + W1/PReLU/W2 + residual
  on rows [512c, 512(c+1)) of x2.

Matmuls run in bf16 (1 cyc/row on the PE regardless of output width;
fp32r drops to 1/4 rate for outputs narrower than 256). LN normalizes
run on the DVE — GpSimd is ~30x slower and was the original bottleneck.
"""

from contextlib import ExitStack

import numpy as np

import concourse.bass as bass
import concourse.tile as tile
from concourse import bacc, mybir
from concourse.bass_utils import run_bass_kernel_spmd
from concourse.masks import make_identity, make_causal_mask

F32 = mybir.dt.float32
F32R = mybir.dt.float32r
BF16 = mybir.dt.bfloat16
# FFN W1/W2/fT/h2T dtype: BF16 halves the dominant 32MB weight stream
# (rel-err impact validated on HW before adoption)
FFN_WDT = BF16
AF = mybir.ActivationFunctionType
ALU = mybir.AluOpType

B, T, C, H, HS = 4, 1024, 1024, 16, 64
NCORES = 8
EPS = 1e-5
SCALE = float(C) ** -0.5  # 1/32, folded into the softmax exp
NEG = -1e30

NTB = T // 128   # 8 token blocks per batch
NCC = C // 128   # 8 channel chunks


# --------------------------------------------------------------------------
# kernel A: attention, 2 heads per core, all batches
# --------------------------------------------------------------------------

def _attn_body(ctx, tc, x, wq, wk, wv, lnw, lnb, catoutT):
    """Transposed-scores attention: scoresT[s,t] with s on partitions.

    softmax denominator comes from an appended ones-column in v (av psum
    column 64), normalization is a per-partition scale on the av output,
    so no wei transposes are needed; only [t,d]->[d,t] cat transposes.

    All matmul operands are bf16 (1 cyc/row on the PE even for <256-wide
    outputs, where fp32r drops to 1/4 rate); LN normalize runs on the DVE
    (GpSimd is ~30x slower and was the launch bottleneck).

    Each matmul pays ~75ns of weight-load overhead, so matmuls are batched
    to the widest moving dim possible: scores in <=512-wide chunks, v via
    a [d,t] projection (512-wide) + per-block PE transposes.

    The per-head outputs are written TRANSPOSED (catT [128 d, B*T]) so
    launch B can feed its Wo projection without any transposes.
    """
    nc = tc.nc

    const = ctx.enter_context(tc.tile_pool(name="const", bufs=1))
    scratch = const.tile([128, 128], F32)
    make_identity(nc, scratch)
    ident = const.tile([128, 128], BF16)
    nc.vector.tensor_copy(out=ident, in_=scratch)
    # transposed causal mask for diagonal blocks: keep s<=t (cols>=rows)
    trilT = const.tile([128, 128], F32)
    nc.gpsimd.memset(trilT, 0.0)
    nc.gpsimd.affine_select(
        out=trilT, in_=trilT, compare_op=ALU.is_ge, fill=NEG, base=0,
        pattern=[[1, 128]], channel_multiplier=-1)
    ones8 = const.tile([128, NTB], BF16)
    nc.vector.memset(ones8, 1.0)
    zero132 = const.tile([128, 132], BF16)
    nc.vector.memset(zero132, 0.0)
    eps_t = const.tile([128, 1], F32)
    nc.vector.memset(eps_t, EPS)

    wq_sb = const.tile([128, NCC, 128], BF16, tag="wq")
    wk_sb = const.tile([128, NCC, 128], BF16, tag="wk")
    wv_sb = const.tile([128, NCC, 128], BF16, tag="wv")
    nc.sync.dma_start(out=wq_sb, in_=wq.rearrange("(cc p) d -> p cc d", p=128))
    nc.sync.dma_start(out=wk_sb, in_=wk.rearrange("(cc p) d -> p cc d", p=128))
    nc.sync.dma_start(out=wv_sb, in_=wv.rearrange("(cc p) d -> p cc d", p=128))
    general_ln = lnw is not None
    if general_ln:
        lnw_bc = const.tile([128, C], F32, tag="lnw")
        lnb_bc = const.tile([128, C], F32, tag="lnb")
        nc.sync.dma_start(
            out=lnw_bc,
            in_=bass.AP(tensor=lnw.tensor, offset=lnw.offset,
                        ap=[[0, 128]] + list(lnw.ap)))
        nc.sync.dma_start(
            out=lnb_bc,
            in_=bass.AP(tensor=lnb.tensor, offset=lnb.offset,
                        ap=[[0, 128]] + list(lnb.ap)))

    xp = ctx.enter_context(tc.tile_pool(name="xp", bufs=5))
    hp = ctx.enter_context(tc.tile_pool(name="hp", bufs=9))
    hTp = ctx.enter_context(tc.tile_pool(name="hTp", bufs=1))
    stat = ctx.enter_context(tc.tile_pool(name="stat", bufs=4))
    qkp = ctx.enter_context(tc.tile_pool(name="qkp", bufs=2))
    vp = ctx.enter_context(tc.tile_pool(name="vp", bufs=2))
    epl = ctx.enter_context(tc.tile_pool(name="epl", bufs=2))
    ctkp = ctx.enter_context(tc.tile_pool(name="ctkp", bufs=10))
    catBp = ctx.enter_context(tc.tile_pool(name="catBp", bufs=2))

    # PSUM banks: mm 2x[128,512]=2, score 2x[128,1024]=4, tr4 2x[128,512]=2
    PSM = ctx.enter_context(tc.tile_pool(name="psm", bufs=2, space="PSUM"))
    PSS = ctx.enter_context(tc.tile_pool(name="pss", bufs=2, space="PSUM"))
    PST = ctx.enter_context(tc.tile_pool(name="pst", bufs=2, space="PSUM"))

    for b in range(B):
        # ---- LN1: rstd batched per group of 4 token tiles ----
        h_tiles = []
        for g in range(2):
            mvs = stat.tile([128, 4, 2], F32, tag="mvs", name=f"mvs_{b}_{g}")
            rstd = stat.tile([128, 4], F32, tag="rstd", name=f"rstd_{b}_{g}")
            lnv = stat.tile([128, 4], F32, tag="lnv", name=f"lnv_{b}_{g}")
            xts = []
            for j in range(4):
                i = g * 4 + j
                r0 = (b * NTB + i) * 128
                xt = xp.tile([128, C], BF16, tag="x", name=f"x_{b}_{i}")
                nc.sync.dma_start(out=xt, in_=x[r0:r0 + 128, :])
                st = stat.tile([128, 2, 6], F32, tag="bn", name=f"bn_{b}_{i}")
                for k in range(2):
                    nc.vector.bn_stats(out=st[:, k, :],
                                       in_=xt[:, k * 512:(k + 1) * 512])
                nc.vector.bn_aggr(out=mvs[:, j, :], in_=st)
                xts.append(xt)
            nc.scalar.activation(out=lnv, in_=mvs[:, :, 1], func=AF.Ln,
                                 bias=eps_t)
            nc.scalar.activation(out=rstd, in_=lnv, func=AF.Exp, scale=-0.5)
            for j in range(4):
                i = g * 4 + j
                ht = hp.tile([128, C], BF16, tag="h", name=f"h_{b}_{i}")
                nc.vector.tensor_scalar(
                    out=ht, in0=xts[j], scalar1=mvs[:, j, 0:1],
                    scalar2=rstd[:, j:j + 1], op0=ALU.subtract, op1=ALU.mult)
                if general_ln:
                    nc.vector.tensor_mul(out=ht, in0=ht, in1=lnw_bc)
                    nc.vector.tensor_add(out=ht, in0=ht, in1=lnb_bc)
                h_tiles.append(ht)

        # ---- transpose h -> hT, grouped 4 blocks per psum/copy ----
        hT = hTp.tile([128, NCC, T], BF16, tag="hT")
        for cc in range(NCC):
            for g in range(2):
                pt = PST.tile([128, 512], BF16, tag="tr4",
                              name=f"pt_{b}_{cc}_{g}")
                for j in range(4):
                    i = g * 4 + j
                    nc.tensor.transpose(
                        pt[:, j * 128:(j + 1) * 128],
                        h_tiles[i][:, cc * 128:(cc + 1) * 128], ident)
                eng = nc.vector if (cc + g) % 2 else nc.scalar
                if eng is nc.scalar:
                    nc.scalar.copy(
                        out=hT[:, cc, g * 512:(g + 1) * 512], in_=pt)
                else:
                    nc.vector.tensor_copy(
                        out=hT[:, cc, g * 512:(g + 1) * 512], in_=pt)

        # ---- qkv (2 heads packed: d2 = 128) ----
        qT2 = qkp.tile([128, T], BF16, tag="qT", name=f"qT_{b}")
        kT2 = qkp.tile([128, T], BF16, tag="kT", name=f"kT_{b}")
        for tch in range(T // 512):
            tsl = slice(tch * 512, (tch + 1) * 512)
            pq = PSM.tile([128, 512], F32, tag="mm", name=f"pq_{b}_{tch}")
            for cc in range(NCC):
                nc.tensor.matmul(pq, wq_sb[:, cc, :], hT[:, cc, tsl],
                                 start=(cc == 0), stop=(cc == NCC - 1))
            nc.scalar.copy(out=qT2[:, tsl], in_=pq)
            pk = PSM.tile([128, 512], F32, tag="mm", name=f"pk_{b}_{tch}")
            for cc in range(NCC):
                nc.tensor.matmul(pk, wk_sb[:, cc, :], hT[:, cc, tsl],
                                 start=(cc == 0), stop=(cc == NCC - 1))
            nc.scalar.copy(out=kT2[:, tsl], in_=pk)
        # vT: [d2, t] via 512-wide streams (same form as q/k)
        vT2 = qkp.tile([128, T], BF16, tag="vT", name=f"vT_{b}")
        for tch in range(T // 512):
            tsl = slice(tch * 512, (tch + 1) * 512)
            pv = PSM.tile([128, 512], F32, tag="mm", name=f"pvT_{b}_{tch}")
            for cc in range(NCC):
                nc.tensor.matmul(pv, wv_sb[:, cc, :], hT[:, cc, tsl],
                                 start=(cc == 0), stop=(cc == NCC - 1))
            nc.scalar.copy(out=vT2[:, tsl], in_=pv)
        # v2: [t_part, sc, 130]: per head 65 cols (64 v + ones), built by
        # transposing vT back to [t, d] in groups of 4 blocks per psum tile
        v2 = vp.tile([128, NTB, 132], BF16, tag="v2", name=f"v2_{b}")
        for i in range(NTB):
            nc.vector.tensor_copy(out=v2[:, i, :], in_=zero132)
            nc.vector.tensor_copy(out=v2[:, i, 64:65], in_=ones8[:, i:i + 1])
            nc.vector.tensor_copy(out=v2[:, i, 130:131], in_=ones8[:, i:i + 1])
        for g in range(2):
            ptv = PST.tile([128, 512], BF16, tag="tr4", name=f"ptv_{b}_{g}")
            for j in range(4):
                i = g * 4 + j
                nc.tensor.transpose(
                    ptv[:, j * 128:(j + 1) * 128],
                    vT2[:, i * 128:(i + 1) * 128], ident)
            for j in range(4):
                i = g * 4 + j
                nc.vector.tensor_copy(out=v2[:, i, 0:64],
                                      in_=ptv[:, j * 128:j * 128 + 64])
                nc.vector.tensor_copy(out=v2[:, i, 66:130],
                                      in_=ptv[:, j * 128 + 64:(j + 1) * 128])

        # ---- attention ----
        cat_toks = [ctkp.tile([128, 128], BF16, tag="ctk",
                              name=f"ctk_{b}_{i}") for i in range(NTB)]
        for h in range(2):
            hsl = slice(h * 64, (h + 1) * 64)
            # scoresT + exp, one psum + one exp per s-chunk
            eps_list = []
            for sc in range(NTB):
                W = (NTB - sc) * 128  # t columns: blocks sc..7
                pss = PSS.tile([128, W], F32, tag="score",
                               name=f"pss_{b}_{h}_{sc}")
                # one matmul per 512-aligned chunk (t cols are contiguous
                # in kT2); per-128-block matmuls waste ~75ns each on
                # weight (re)loads of the same qT block
                for n0 in range(0, W, 512):
                    n1 = min(n0 + 512, W)
                    nc.tensor.matmul(
                        pss[:, n0:n1],
                        qT2[hsl, sc * 128:(sc + 1) * 128],
                        kT2[hsl, sc * 128 + n0:sc * 128 + n1],
                        start=True, stop=True)
                nc.vector.tensor_add(out=pss[:, 0:128], in0=pss[:, 0:128],
                                     in1=trilT)
                e_sc = epl.tile([128, W], BF16, tag=f"e{sc}",
                                name=f"e_{b}_{h}_{sc}")
                # one exp op per PSUM bank (bank-crossing ACT reads are
                # suspect for the NRT_EXEC_UNIT_UNRECOVERABLE wedge)
                n0 = 0
                while n0 < W:
                    n1 = min(n0 + 512, W)
                    nc.scalar.activation(out=e_sc[:, n0:n1],
                                         in_=pss[:, n0:n1], func=AF.Exp,
                                         scale=SCALE)
                    n0 = n1
                eps_list.append(e_sc)
            # av + normalize into cat_tok
            for i in range(NTB):
                po = PSM.tile([128, 66], F32, tag="mm",
                              name=f"po_{b}_{h}_{i}")
                for sc in range(i + 1):
                    j = i - sc
                    nc.tensor.matmul(
                        po, eps_list[sc][:, j * 128:(j + 1) * 128],
                        v2[:, sc, h * 66:(h + 1) * 66],
                        start=(sc == 0), stop=(sc == i))
                rec = stat.tile([128, 1], F32, tag="rec",
                                name=f"rec_{b}_{h}_{i}")
                nc.vector.reciprocal(out=rec, in_=po[:, 64:65])
                nc.vector.tensor_scalar_mul(
                    out=cat_toks[i][:, hsl], in0=po[:, 0:64], scalar1=rec)

        # ---- transpose cat -> [d, t] and write catT to DRAM ----
        # (launch B consumes catT directly for the Wo projection, so it
        # does no transposes and starts its matmuls immediately)
        for g in range(2):
            ptc = PST.tile([128, 512], BF16, tag="tr4", name=f"ptc_{b}_{g}")
            for j in range(4):
                nc.tensor.transpose(
                    ptc[:, j * 128:(j + 1) * 128], cat_toks[g * 4 + j], ident)
            catB = catBp.tile([128, 512], BF16, tag="catB",
                              name=f"catB_{b}_{g}")
            nc.vector.tensor_copy(out=catB, in_=ptc)
            c0 = b * T + g * 512
            nc.sync.dma_start(out=catoutT[:, c0:c0 + 512], in_=catB)


def _build_attn(general_ln: bool, repeat: int = 1):
    nc = bacc.Bacc("TRN2", target_bir_lowering=False, debug=False)
    x = nc.dram_tensor("x", [B * T, C], BF16, kind="ExternalInput").ap()
    wq = nc.dram_tensor("wq", [C, 128], BF16, kind="ExternalInput").ap()
    wk = nc.dram_tensor("wk", [C, 128], BF16, kind="ExternalInput").ap()
    wv = nc.dram_tensor("wv", [C, 128], BF16, kind="ExternalInput").ap()
    lnw = lnb = None
    if general_ln:
        lnw = nc.dram_tensor("lnw", [C], F32, kind="ExternalInput").ap()
        lnb = nc.dram_tensor("lnb", [C], F32, kind="ExternalInput").ap()
    catoutT = nc.dram_tensor("catT", [128, B * T], BF16,
                             kind="ExternalOutput").ap()
    with tile.TileContext(nc) as tc:
        for _ in range(repeat):
            with ExitStack() as ctx:
                _attn_body(ctx, tc, x, wq, wk, wv, lnw, lnb, catoutT)
    nc.compile()
    return nc


# --------------------------------------------------------------------------
# kernel B: FFN, 512 rows per core
# --------------------------------------------------------------------------

RPC = (B * T) // NCORES  # 512 rows per core
NRB = RPC // 128         # 4 row blocks
NHID = 4 * C // 128      # 32 hidden chunks


def _ffn_body(ctx, tc, xr, catT, wo, w1, w2, bo, b1, ln2w, ln2b, b2,
              alpha, out, wdt=F32R):
    """Per-core rows: proj = catT.T @ Wo (+bo); x2 = x + proj; LN2 + FFN.

    catT arrives pre-transposed from launch A, so the projection is pure
    matmul and pipelines per row-block with LN2/h2T. PReLU is a single
    scalar-engine Lrelu op. wdt: dtype for W1/W2/fT/h2T.
    """
    nc = tc.nc
    general_ln = ln2w is not None

    const = ctx.enter_context(tc.tile_pool(name="const", bufs=1))
    scratch = const.tile([128, 128], F32)
    make_identity(nc, scratch)
    ident = const.tile([128, 128], BF16)
    nc.vector.tensor_copy(out=ident, in_=scratch)
    eps_t = const.tile([128, 1], F32)
    nc.vector.memset(eps_t, EPS)
    b1_sb = None
    if b1 is not None:
        b1_sb = const.tile([128, NHID], F32, tag="b1")
        nc.sync.dma_start(out=b1_sb, in_=b1.rearrange("(h p) -> p h", p=128))

    def bcast(src, tag):
        t = const.tile([128, C], F32, tag=tag, name=tag)
        nc.sync.dma_start(
            out=t, in_=bass.AP(tensor=src.tensor, offset=src.offset,
                               ap=[[0, 128]] + list(src.ap)))
        return t

    bo_bc = bcast(bo, "bo") if bo is not None else None
    lnw_bc = bcast(ln2w, "lnw") if general_ln else None
    lnb_bc = bcast(ln2b, "lnb") if general_ln else None
    b2_bc = bcast(b2, "b2") if b2 is not None else None

    wo_sb = const.tile([128, NCC, C], BF16, tag="wo")
    nc.sync.dma_start(out=wo_sb, in_=wo.rearrange("(cc p) c -> p cc c", p=128))
    ctT = const.tile([128, NCC, RPC], BF16, tag="catT")
    nc.sync.dma_start(out=ctT, in_=catT.rearrange("(cc p) t -> p cc t", p=128))

    xrp = ctx.enter_context(tc.tile_pool(name="xrp", bufs=2))
    x2p = ctx.enter_context(tc.tile_pool(name="x2p", bufs=NRB))
    hp = ctx.enter_context(tc.tile_pool(name="hp", bufs=2))
    h2Tp = ctx.enter_context(tc.tile_pool(name="h2Tp", bufs=1))
    stat = ctx.enter_context(tc.tile_pool(name="stat", bufs=8))
    w1p = ctx.enter_context(tc.tile_pool(name="w1p", bufs=4))
    w2p = ctx.enter_context(tc.tile_pool(name="w2p", bufs=4))
    ftp = ctx.enter_context(tc.tile_pool(name="ftp", bufs=NHID))
    osb = ctx.enter_context(tc.tile_pool(name="osb", bufs=2))

    x2_tiles = []
    h2T = h2Tp.tile([128, NCC, RPC], wdt, tag="h2T")
    with tc.tile_pool(name="psp", bufs=2, space="PSUM") as PSP, \
         tc.tile_pool(name="pst", bufs=2, space="PSUM") as PST:
        # ---- proj + residual + LN2 + transpose, pipelined per r ----
        for r in range(NRB):
            rsl = slice(r * 128, (r + 1) * 128)
            xt = xrp.tile([128, C], F32, tag="xr", name=f"xr_{r}")
            nc.sync.dma_start(out=xt, in_=xr[rsl, :])
            pps = PSP.tile([128, C], F32, tag="pp", name=f"pp_{r}")
            for cc in range(NCC):
                for co in range(2):
                    csl = slice(co * 512, (co + 1) * 512)
                    nc.tensor.matmul(pps[:, csl], ctT[:, cc, rsl],
                                     wo_sb[:, cc, csl],
                                     start=(cc == 0), stop=(cc == NCC - 1))
            x2t = x2p.tile([128, C], F32, tag="x2", name=f"x2_{r}")
            nc.vector.tensor_add(out=x2t, in0=pps, in1=xt)
            if bo_bc is not None:
                nc.vector.tensor_add(out=x2t, in0=x2t, in1=bo_bc)
            x2_tiles.append(x2t)
            # LN2 on this row block
            st = stat.tile([128, 2, 6], F32, tag="bn", name=f"bn_{r}")
            for k in range(2):
                nc.vector.bn_stats(out=st[:, k, :],
                                   in_=x2t[:, k * 512:(k + 1) * 512])
            mv = stat.tile([128, 2], F32, tag="mv", name=f"mv_{r}")
            nc.vector.bn_aggr(out=mv, in_=st)
            lnv = stat.tile([128, 1], F32, tag="lnv", name=f"lnv_{r}")
            nc.scalar.activation(out=lnv, in_=mv[:, 1:2], func=AF.Ln,
                                 bias=eps_t)
            rstd = stat.tile([128, 1], F32, tag="rstd", name=f"rstd_{r}")
            nc.scalar.activation(out=rstd, in_=lnv, func=AF.Exp, scale=-0.5)
            ht = hp.tile([128, C], BF16, tag="h", name=f"h_{r}")
            nc.vector.tensor_scalar(
                out=ht, in0=x2t, scalar1=mv[:, 0:1], scalar2=rstd,
                op0=ALU.subtract, op1=ALU.mult)
            if general_ln:
                nc.vector.tensor_mul(out=ht, in0=ht, in1=lnw_bc)
                nc.vector.tensor_add(out=ht, in0=ht, in1=lnb_bc)
            for g in range(2):
                pt = PST.tile([128, 4, 128], BF16, tag="tr4",
                              name=f"pt_{r}_{g}")
                for j in range(4):
                    cc = g * 4 + j
                    nc.tensor.transpose(pt[:, j, :],
                                        ht[:, cc * 128:(cc + 1) * 128], ident)
                nc.scalar.copy(out=h2T[:, g * 4:(g + 1) * 4, rsl], in_=pt)

    with tc.tile_pool(name="psf", bufs=2, space="PSUM") as PSF:
        # ---- phase 1: fT[h] = Lrelu(W1_h^T @ h2 + b1) ----
        f_tiles = []
        w1r = w1.rearrange("(cc p) (h q) -> p cc h q", p=128, q=128)
        for h in range(NHID):
            w1_sb = w1p.tile([128, NCC, 128], wdt, tag="w1",
                             name=f"w1_{h}")
            nc.sync.dma_start(out=w1_sb, in_=w1r[:, :, h, :])
            pf = PSF.tile([128, RPC], F32, tag="ft", name=f"pf_{h}")
            for cc in range(NCC):
                nc.tensor.matmul(pf, w1_sb[:, cc, :], h2T[:, cc, :],
                                 start=(cc == 0), stop=(cc == NCC - 1))
            ft = ftp.tile([128, RPC], wdt, tag="ft", name=f"ft_{h}")
            if b1_sb is not None:
                nc.scalar.activation(out=ft, in_=pf, func=AF.Lrelu,
                                     bias=b1_sb[:, h:h + 1], alpha=alpha)
            else:
                nc.scalar.activation(out=ft, in_=pf, func=AF.Lrelu,
                                     alpha=alpha)
            f_tiles.append(ft)

    # ---- phase 2: out = fT.T @ W2 (+b2) + x2 ----
    with tc.tile_pool(name="pso", bufs=NRB, space="PSUM") as PSO:
        pouts = [PSO.tile([128, C], F32, tag="out", name=f"pout{r}")
                 for r in range(NRB)]
        for h in range(NHID):
            w2_sb = w2p.tile([128, C], wdt, tag="w2", name=f"w2_{h}")
            nc.sync.dma_start(out=w2_sb, in_=w2[h * 128:(h + 1) * 128, :])
            for r in range(NRB):
                for co in range(2):
                    csl = slice(co * 512, (co + 1) * 512)
                    nc.tensor.matmul(pouts[r][:, csl],
                                     f_tiles[h][:, r * 128:(r + 1) * 128],
                                     w2_sb[:, csl],
                                     start=(h == 0), stop=(h == NHID - 1))
        for r in range(NRB):
            o_sb = osb.tile([128, C], F32, tag="o", name=f"o_{r}")
            nc.vector.tensor_add(out=o_sb, in0=pouts[r], in1=x2_tiles[r])
            if b2_bc is not None:
                nc.vector.tensor_add(out=o_sb, in0=o_sb, in1=b2_bc)
            nc.sync.dma_start(out=out[r * 128:(r + 1) * 128, :], in_=o_sb)


def _build_ffn(general_ln: bool, has_bo: bool, has_b1: bool, has_b2: bool,
               alpha: float, repeat: int = 1, wdt=F32R):
    nc = bacc.Bacc("TRN2", target_bir_lowering=False, debug=False)
    xr = nc.dram_tensor("xr", [RPC, C], F32, kind="ExternalInput").ap()
    catT = nc.dram_tensor("catT", [C, RPC], BF16, kind="ExternalInput").ap()
    wo = nc.dram_tensor("wo", [C, C], BF16, kind="ExternalInput").ap()
    w1 = nc.dram_tensor("w1", [C, 4 * C], wdt, kind="ExternalInput").ap()
    w2 = nc.dram_tensor("w2", [4 * C, C], wdt, kind="ExternalInput").ap()
    bo = b1 = ln2w = ln2b = b2 = None
    if has_bo:
        bo = nc.dram_tensor("bo", [C], F32, kind="ExternalInput").ap()
    if has_b1:
        b1 = nc.dram_tensor("b1", [4 * C], F32, kind="ExternalInput").ap()
    if general_ln:
        ln2w = nc.dram_tensor("ln2w", [C], F32, kind="ExternalInput").ap()
        ln2b = nc.dram_tensor("ln2b", [C], F32, kind="ExternalInput").ap()
    if has_b2:
        b2 = nc.dram_tensor("b2", [C], F32, kind="ExternalInput").ap()
    out = nc.dram_tensor("out", [RPC, C], F32, kind="ExternalOutput").ap()
    with tile.TileContext(nc) as tc:
        for _ in range(repeat):
            with ExitStack() as ctx:
                _ffn_body(ctx, tc, xr, catT, wo, w1, w2, bo, b1,
                          ln2w, ln2b, b2, alpha, out, wdt=wdt)
    nc.compile()
    return nc


# --------------------------------------------------------------------------
# host orchestration
# --------------------------------------------------------------------------

_NC_CACHE = {}


def _get_attn_nc(general_ln):
    key = ("attn", general_ln)
    if key not in _NC_CACHE:
        _NC_CACHE[key] = _build_attn(general_ln)
    return _NC_CACHE[key]


def _get_ffn_nc(general_ln, has_bo, has_b1, has_b2, alpha, wdt=None):
    wdt = FFN_WDT if wdt is None else wdt
    key = ("ffn", general_ln, has_bo, has_b1, has_b2, float(alpha), wdt)
    if key not in _NC_CACHE:
        _NC_CACHE[key] = _build_ffn(general_ln, has_bo, has_b1, has_b2,
                                    float(alpha), wdt=wdt)
    return _NC_CACHE[key]


def _w_np(a):
    if FFN_WDT == BF16:
        import ml_dtypes
        return np.ascontiguousarray(a.astype(ml_dtypes.bfloat16))
    return a


def _bf(a):
    import ml_dtypes
    return np.ascontiguousarray(np.asarray(a).astype(ml_dtypes.bfloat16))


def attn_in_maps(x_flat, Wq, Wk, Wv, trivial, ln1_w, ln1_b):
    x_bf = _bf(x_flat)
    in_maps = []
    for c in range(NCORES):
        h0 = 2 * c
        m = {
            "x": x_bf,
            "wq": _bf(np.concatenate([Wq[h0], Wq[h0 + 1]], axis=1)),
            "wk": _bf(np.concatenate([Wk[h0], Wk[h0 + 1]], axis=1)),
            "wv": _bf(np.concatenate([Wv[h0], Wv[h0 + 1]], axis=1)),
        }
        if not trivial:
            m["lnw"] = ln1_w
            m["lnb"] = ln1_b
        in_maps.append(m)
    return in_maps


def run_attn(x_flat, Wq, Wk, Wv, ln1_w, ln1_b):
    """Returns catT [C, B*T] bf16: transposed per-head attention outputs."""
    trivial = bool(np.all(ln1_w == 1.0) and np.all(ln1_b == 0.0))
    nc = _get_attn_nc(not trivial)
    in_maps = attn_in_maps(x_flat, Wq, Wk, Wv, trivial, ln1_w, ln1_b)
    res = run_bass_kernel_spmd(nc, in_maps, list(range(NCORES)), trace=False)
    return np.concatenate(
        [res.results[c]["catT"] for c in range(NCORES)], axis=0)


def ffn_in_maps(x_flat, catT_all, Wo, bo, W1, b1, W2, b2, ln2_w, ln2_b,
                flags):
    trivial, has_bo, has_b1, has_b2 = flags
    wo_np = _bf(Wo)
    w1_np, w2_np = _w_np(W1), _w_np(W2)
    in_maps = []
    for c in range(NCORES):
        sl = slice(RPC * c, RPC * (c + 1))
        m = {
            "xr": np.ascontiguousarray(x_flat[sl]),
            "catT": np.ascontiguousarray(catT_all[:, sl]),
            "wo": wo_np,
            "w1": w1_np,
            "w2": w2_np,
        }
        if has_bo:
            m["bo"] = bo
        if has_b1:
            m["b1"] = b1
        if not trivial:
            m["ln2w"] = ln2_w
            m["ln2b"] = ln2_b
        if has_b2:
            m["b2"] = b2
        in_maps.append(m)
    return in_maps


def run_ffn(x_flat, catT_all, Wo, bo, W1, b1, W2, b2, ln2_w, ln2_b, alpha):
    trivial = bool(np.all(ln2_w == 1.0) and np.all(ln2_b == 0.0))
    has_bo = bool(np.any(bo != 0.0))
    has_b1 = bool(np.any(b1 != 0.0))
    has_b2 = bool(np.any(b2 != 0.0))
    nc = _get_ffn_nc(not trivial, has_bo, has_b1, has_b2, alpha)
    flags = (trivial, has_bo, has_b1, has_b2)
    in_maps = ffn_in_maps(x_flat, catT_all, Wo, bo, W1, b1, W2, b2,
                          ln2_w, ln2_b, flags)
    res = run_bass_kernel_spmd(nc, in_maps, list(range(NCORES)), trace=False)
    return np.concatenate(
        [res.results[c]["out"] for c in range(NCORES)], axis=0)


def kernel(x, ln1_w, ln1_b, Wk, Wq, Wv, Wo, bo, ln2_w, ln2_b, W1, b1,
           prelu_a, W2, b2):
    x = np.asarray(x, np.float32)
    x_flat = np.ascontiguousarray(x.reshape(B * T, C))
    Wq = np.asarray(Wq, np.float32)
    Wk = np.asarray(Wk, np.float32)
    Wv = np.asarray(Wv, np.float32)
    Wo = np.asarray(Wo, np.float32)
    alpha = float(np.asarray(prelu_a))

    catT_all = run_attn(x_flat, Wq, Wk, Wv,
                        np.asarray(ln1_w, np.float32),
                        np.asarray(ln1_b, np.float32))
    out = run_ffn(x_flat, catT_all, Wo, np.asarray(bo, np.float32),
                  np.asarray(W1, np.float32), np.asarray(b1, np.float32),
                  np.asarray(W2, np.float32), np.asarray(b2, np.float32),
                  np.asarray(ln2_w, np.float32),
                  np.asarray(ln2_b, np.float32), alpha)
    return out.reshape(B, T, C).astype(np.float32)



# revision 41
# speedup vs baseline: 1.1786x; 1.1786x over previous
"""Trainium2 Bass kernel for a dense pre-LN transformer block.

B=4, T=1024, C=1024, H=16 heads (head_size 64).

Distribution over the 8 NeuronCores (two SPMD launches, host-side
reduction between them):

  Launch A (attention, head-parallel): every core runs the identical
  program on all 4 batches but with its own pair of heads (weight
  slices are per-core input data). Each core computes LN1, its 2 heads'
  q/k/v + causal attention, and the partial Wo projection of those
  heads for the whole [B*T, C] output (written f32 from PSUM).
  NOTE the reference computes scores as k @ q^T (roles of q/k swapped
  vs standard attention) — handled by using k rows as the "queries".

  Host: x2 = x + sum_c projpart_c + bo.

  Launch B (FFN, row-parallel): core c runs LN2 + W1/PReLU/W2 + residual
  on rows [512c, 512(c+1)) of x2.

Matmuls run in bf16 (1 cyc/row on the PE regardless of output width;
fp32r drops to 1/4 rate for outputs narrower than 256). LN normalizes
run on the DVE — GpSimd is ~30x slower and was the original bottleneck.
"""

from contextlib import ExitStack

import numpy as np

import concourse.bass as bass
import concourse.tile as tile
from concourse import bacc, mybir
from concourse.bass_utils import run_bass_kernel_spmd
from concourse.masks import make_identity, make_causal_mask

F32 = mybir.dt.float32
F32R = mybir.dt.float32r
BF16 = mybir.dt.bfloat16
# FFN W1/W2/fT/h2T dtype: BF16 halves the dominant 32MB weight stream
# (rel-err impact validated on HW before adoption)
FFN_WDT = BF16
AF = mybir.ActivationFunctionType
ALU = mybir.AluOpType

B, T, C, H, HS = 4, 1024, 1024, 16, 64
NCORES = 8
EPS = 1e-5
SCALE = float(C) ** -0.5  # 1/32, folded into the softmax exp
NEG = -1e30

NTB = T // 128   # 8 token blocks per batch
NCC = C // 128   # 8 channel chunks


# --------------------------------------------------------------------------
# kernel A: attention, 2 heads per core, all batches
# --------------------------------------------------------------------------

def _attn_body(ctx, tc, x, wq, wk, wv, lnw, lnb, catoutT):
    """Transposed-scores attention: scoresT[s,t] with s on partitions.

    softmax denominator comes from an appended ones-column in v (av psum
    column 64), normalization is a per-partition scale on the av output,
    so no wei transposes are needed; only [t,d]->[d,t] cat transposes.

    All matmul operands are bf16 (1 cyc/row on the PE even for <256-wide
    outputs, where fp32r drops to 1/4 rate); LN normalize runs on the DVE
    (GpSimd is ~30x slower and was the launch bottleneck).

    Each matmul pays ~75ns of weight-load overhead, so matmuls are batched
    to the widest moving dim possible: scores in <=512-wide chunks, v via
    a [d,t] projection (512-wide) + per-block PE transposes.

    The per-head outputs are written TRANSPOSED (catT [128 d, B*T]) so
    launch B can feed its Wo projection without any transposes.
    """
    nc = tc.nc

    const = ctx.enter_context(tc.tile_pool(name="const", bufs=1))
    scratch = const.tile([128, 128], F32)
    make_identity(nc, scratch)
    ident = const.tile([128, 128], BF16)
    nc.vector.tensor_copy(out=ident, in_=scratch)
    # transposed causal mask for diagonal blocks: keep s<=t (cols>=rows)
    trilT = const.tile([128, 128], F32)
    nc.gpsimd.memset(trilT, 0.0)
    nc.gpsimd.affine_select(
        out=trilT, in_=trilT, compare_op=ALU.is_ge, fill=NEG, base=0,
        pattern=[[1, 128]], channel_multiplier=-1)
    ones8 = const.tile([128, NTB], BF16)
    nc.vector.memset(ones8, 1.0)
    zero132 = const.tile([128, 132], BF16)
    nc.vector.memset(zero132, 0.0)
    eps_t = const.tile([128, 1], F32)
    nc.vector.memset(eps_t, EPS)

    wq_sb = const.tile([128, NCC, 128], BF16, tag="wq")
    wk_sb = const.tile([128, NCC, 128], BF16, tag="wk")
    wv_sb = const.tile([128, NCC, 128], BF16, tag="wv")
    nc.sync.dma_start(out=wq_sb, in_=wq.rearrange("(cc p) d -> p cc d", p=128))
    nc.sync.dma_start(out=wk_sb, in_=wk.rearrange("(cc p) d -> p cc d", p=128))
    nc.sync.dma_start(out=wv_sb, in_=wv.rearrange("(cc p) d -> p cc d", p=128))
    general_ln = lnw is not None
    if general_ln:
        lnw_bc = const.tile([128, C], F32, tag="lnw")
        lnb_bc = const.tile([128, C], F32, tag="lnb")
        nc.sync.dma_start(
            out=lnw_bc,
            in_=bass.AP(tensor=lnw.tensor, offset=lnw.offset,
                        ap=[[0, 128]] + list(lnw.ap)))
        nc.sync.dma_start(
            out=lnb_bc,
            in_=bass.AP(tensor=lnb.tensor, offset=lnb.offset,
                        ap=[[0, 128]] + list(lnb.ap)))

    xp = ctx.enter_context(tc.tile_pool(name="xp", bufs=5))
    hp = ctx.enter_context(tc.tile_pool(name="hp", bufs=9))
    hTp = ctx.enter_context(tc.tile_pool(name="hTp", bufs=1))
    stat = ctx.enter_context(tc.tile_pool(name="stat", bufs=4))
    qkp = ctx.enter_context(tc.tile_pool(name="qkp", bufs=2))
    vp = ctx.enter_context(tc.tile_pool(name="vp", bufs=2))
    epl = ctx.enter_context(tc.tile_pool(name="epl", bufs=2))
    ctkp = ctx.enter_context(tc.tile_pool(name="ctkp", bufs=10))
    catBp = ctx.enter_context(tc.tile_pool(name="catBp", bufs=2))

    # PSUM banks: mm 2x[128,512]=2, score 2x[128,1024]=4, tr4 2x[128,512]=2
    PSM = ctx.enter_context(tc.tile_pool(name="psm", bufs=2, space="PSUM"))
    PSS = ctx.enter_context(tc.tile_pool(name="pss", bufs=2, space="PSUM"))
    PST = ctx.enter_context(tc.tile_pool(name="pst", bufs=2, space="PSUM"))

    for b in range(B):
        # ---- LN1: rstd batched per group of 4 token tiles ----
        h_tiles = []
        for g in range(2):
            mvs = stat.tile([128, 4, 2], F32, tag="mvs", name=f"mvs_{b}_{g}")
            rstd = stat.tile([128, 4], F32, tag="rstd", name=f"rstd_{b}_{g}")
            lnv = stat.tile([128, 4], F32, tag="lnv", name=f"lnv_{b}_{g}")
            xts = []
            for j in range(4):
                i = g * 4 + j
                r0 = (b * NTB + i) * 128
                xt = xp.tile([128, C], BF16, tag="x", name=f"x_{b}_{i}")
                nc.sync.dma_start(out=xt, in_=x[r0:r0 + 128, :])
                st = stat.tile([128, 2, 6], F32, tag="bn", name=f"bn_{b}_{i}")
                for k in range(2):
                    nc.vector.bn_stats(out=st[:, k, :],
                                       in_=xt[:, k * 512:(k + 1) * 512])
                nc.vector.bn_aggr(out=mvs[:, j, :], in_=st)
                xts.append(xt)
            nc.scalar.activation(out=lnv, in_=mvs[:, :, 1], func=AF.Ln,
                                 bias=eps_t)
            nc.scalar.activation(out=rstd, in_=lnv, func=AF.Exp, scale=-0.5)
            for j in range(4):
                i = g * 4 + j
                ht = hp.tile([128, C], BF16, tag="h", name=f"h_{b}_{i}")
                nc.vector.tensor_scalar(
                    out=ht, in0=xts[j], scalar1=mvs[:, j, 0:1],
                    scalar2=rstd[:, j:j + 1], op0=ALU.subtract, op1=ALU.mult)
                if general_ln:
                    nc.vector.tensor_mul(out=ht, in0=ht, in1=lnw_bc)
                    nc.vector.tensor_add(out=ht, in0=ht, in1=lnb_bc)
                h_tiles.append(ht)

        # ---- transpose h -> hT, grouped 4 blocks per psum/copy ----
        hT = hTp.tile([128, NCC, T], BF16, tag="hT")
        for cc in range(NCC):
            for g in range(2):
                pt = PST.tile([128, 512], BF16, tag="tr4",
                              name=f"pt_{b}_{cc}_{g}")
                for j in range(4):
                    i = g * 4 + j
                    nc.tensor.transpose(
                        pt[:, j * 128:(j + 1) * 128],
                        h_tiles[i][:, cc * 128:(cc + 1) * 128], ident)
                eng = nc.vector if (cc + g) % 2 else nc.scalar
                if eng is nc.scalar:
                    nc.scalar.copy(
                        out=hT[:, cc, g * 512:(g + 1) * 512], in_=pt)
                else:
                    nc.vector.tensor_copy(
                        out=hT[:, cc, g * 512:(g + 1) * 512], in_=pt)

        # ---- qkv (2 heads packed: d2 = 128) ----
        qT2 = qkp.tile([128, T], BF16, tag="qT", name=f"qT_{b}")
        kT2 = qkp.tile([128, T], BF16, tag="kT", name=f"kT_{b}")
        for tch in range(T // 512):
            tsl = slice(tch * 512, (tch + 1) * 512)
            pq = PSM.tile([128, 512], F32, tag="mm", name=f"pq_{b}_{tch}")
            for cc in range(NCC):
                nc.tensor.matmul(pq, wq_sb[:, cc, :], hT[:, cc, tsl],
                                 start=(cc == 0), stop=(cc == NCC - 1))
            nc.scalar.copy(out=qT2[:, tsl], in_=pq)
            pk = PSM.tile([128, 512], F32, tag="mm", name=f"pk_{b}_{tch}")
            for cc in range(NCC):
                nc.tensor.matmul(pk, wk_sb[:, cc, :], hT[:, cc, tsl],
                                 start=(cc == 0), stop=(cc == NCC - 1))
            nc.scalar.copy(out=kT2[:, tsl], in_=pk)
        # vT: [d2, t] via 512-wide streams (same form as q/k)
        vT2 = qkp.tile([128, T], BF16, tag="vT", name=f"vT_{b}")
        for tch in range(T // 512):
            tsl = slice(tch * 512, (tch + 1) * 512)
            pv = PSM.tile([128, 512], F32, tag="mm", name=f"pvT_{b}_{tch}")
            for cc in range(NCC):
                nc.tensor.matmul(pv, wv_sb[:, cc, :], hT[:, cc, tsl],
                                 start=(cc == 0), stop=(cc == NCC - 1))
            nc.scalar.copy(out=vT2[:, tsl], in_=pv)
        # v2: [t_part, sc, 130]: per head 65 cols (64 v + ones), built by
        # transposing vT back to [t, d] in groups of 4 blocks per psum tile
        v2 = vp.tile([128, NTB, 132], BF16, tag="v2", name=f"v2_{b}")
        for i in range(NTB):
            nc.vector.tensor_copy(out=v2[:, i, :], in_=zero132)
            nc.vector.tensor_copy(out=v2[:, i, 64:65], in_=ones8[:, i:i + 1])
            nc.vector.tensor_copy(out=v2[:, i, 130:131], in_=ones8[:, i:i + 1])
        for g in range(2):
            ptv = PST.tile([128, 512], BF16, tag="tr4", name=f"ptv_{b}_{g}")
            for j in range(4):
                i = g * 4 + j
                nc.tensor.transpose(
                    ptv[:, j * 128:(j + 1) * 128],
                    vT2[:, i * 128:(i + 1) * 128], ident)
            for j in range(4):
                i = g * 4 + j
                nc.vector.tensor_copy(out=v2[:, i, 0:64],
                                      in_=ptv[:, j * 128:j * 128 + 64])
                nc.vector.tensor_copy(out=v2[:, i, 66:130],
                                      in_=ptv[:, j * 128 + 64:(j + 1) * 128])

        # ---- attention ----
        cat_toks = [ctkp.tile([128, 128], BF16, tag="ctk",
                              name=f"ctk_{b}_{i}") for i in range(NTB)]
        for h in range(2):
            hsl = slice(h * 64, (h + 1) * 64)
            # scoresT + exp, one psum + one exp per s-chunk
            eps_list = []
            for sc in range(NTB):
                W = (NTB - sc) * 128  # t columns: blocks sc..7
                pss = PSS.tile([128, W], F32, tag="score",
                               name=f"pss_{b}_{h}_{sc}")
                # one matmul per 512-aligned chunk (t cols are contiguous
                # in kT2); per-128-block matmuls waste ~75ns each on
                # weight (re)loads of the same qT block
                for n0 in range(0, W, 512):
                    n1 = min(n0 + 512, W)
                    nc.tensor.matmul(
                        pss[:, n0:n1],
                        qT2[hsl, sc * 128:(sc + 1) * 128],
                        kT2[hsl, sc * 128 + n0:sc * 128 + n1],
                        start=True, stop=True)
                nc.vector.tensor_add(out=pss[:, 0:128], in0=pss[:, 0:128],
                                     in1=trilT)
                e_sc = epl.tile([128, W], BF16, tag=f"e{sc}",
                                name=f"e_{b}_{h}_{sc}")
                # one exp op per PSUM bank (bank-crossing ACT reads are
                # suspect for the NRT_EXEC_UNIT_UNRECOVERABLE wedge)
                n0 = 0
                while n0 < W:
                    n1 = min(n0 + 512, W)
                    nc.scalar.activation(out=e_sc[:, n0:n1],
                                         in_=pss[:, n0:n1], func=AF.Exp,
                                         scale=SCALE)
                    n0 = n1
                eps_list.append(e_sc)
            # av + normalize into cat_tok
            for i in range(NTB):
                po = PSM.tile([128, 66], F32, tag="mm",
                              name=f"po_{b}_{h}_{i}")
                for sc in range(i + 1):
                    j = i - sc
                    nc.tensor.matmul(
                        po, eps_list[sc][:, j * 128:(j + 1) * 128],
                        v2[:, sc, h * 66:(h + 1) * 66],
                        start=(sc == 0), stop=(sc == i))
                rec = stat.tile([128, 1], F32, tag="rec",
                                name=f"rec_{b}_{h}_{i}")
                nc.vector.reciprocal(out=rec, in_=po[:, 64:65])
                nc.vector.tensor_scalar_mul(
                    out=cat_toks[i][:, hsl], in0=po[:, 0:64], scalar1=rec)

        # ---- transpose cat -> [d, t] and write catT to DRAM ----
        # (launch B consumes catT directly for the Wo projection, so it
        # does no transposes and starts its matmuls immediately)
        for g in range(2):
            ptc = PST.tile([128, 512], BF16, tag="tr4", name=f"ptc_{b}_{g}")
            for j in range(4):
                nc.tensor.transpose(
                    ptc[:, j * 128:(j + 1) * 128], cat_toks[g * 4 + j], ident)
            catB = catBp.tile([128, 512], BF16, tag="catB",
                              name=f"catB_{b}_{g}")
            nc.vector.tensor_copy(out=catB, in_=ptc)
            c0 = b * T + g * 512
            nc.sync.dma_start(out=catoutT[:, c0:c0 + 512], in_=catB)


def _build_attn(general_ln: bool, repeat: int = 1):
    nc = bacc.Bacc("TRN2", target_bir_lowering=False, debug=False)
    x = nc.dram_tensor("x", [B * T, C], BF16, kind="ExternalInput").ap()
    wq = nc.dram_tensor("wq", [C, 128], BF16, kind="ExternalInput").ap()
    wk = nc.dram_tensor("wk", [C, 128], BF16, kind="ExternalInput").ap()
    wv = nc.dram_tensor("wv", [C, 128], BF16, kind="ExternalInput").ap()
    lnw = lnb = None
    if general_ln:
        lnw = nc.dram_tensor("lnw", [C], F32, kind="ExternalInput").ap()
        lnb = nc.dram_tensor("lnb", [C], F32, kind="ExternalInput").ap()
    catoutT = nc.dram_tensor("catT", [128, B * T], BF16,
                             kind="ExternalOutput").ap()
    with tile.TileContext(nc) as tc:
        for _ in range(repeat):
            with ExitStack() as ctx:
                _attn_body(ctx, tc, x, wq, wk, wv, lnw, lnb, catoutT)
    nc.compile()
    return nc


# --------------------------------------------------------------------------
# kernel B: FFN, 512 rows per core
# --------------------------------------------------------------------------

RPC = (B * T) // NCORES  # 512 rows per core
NRB = RPC // 128         # 4 row blocks
NHID = 4 * C // 128      # 32 hidden chunks


def _ffn_body(ctx, tc, xr, catT, wo, w1, w2, bo, b1, ln2w, ln2b, b2,
              alpha, out, wdt=F32R):
    """Per-core rows: proj = catT.T @ Wo (+bo); x2 = x + proj; LN2 + FFN.

    catT arrives pre-transposed from launch A, so the projection is pure
    matmul and pipelines per row-block with LN2/h2T. PReLU is a single
    scalar-engine Lrelu op. wdt: dtype for W1/W2/fT/h2T.
    """
    nc = tc.nc
    general_ln = ln2w is not None

    const = ctx.enter_context(tc.tile_pool(name="const", bufs=1))
    scratch = const.tile([128, 128], F32)
    make_identity(nc, scratch)
    ident = const.tile([128, 128], BF16)
    nc.vector.tensor_copy(out=ident, in_=scratch)
    eps_t = const.tile([128, 1], F32)
    nc.vector.memset(eps_t, EPS)
    b1_sb = None
    if b1 is not None:
        b1_sb = const.tile([128, NHID], F32, tag="b1")
        nc.sync.dma_start(out=b1_sb, in_=b1.rearrange("(h p) -> p h", p=128))

    def bcast(src, tag):
        t = const.tile([128, C], F32, tag=tag, name=tag)
        nc.sync.dma_start(
            out=t, in_=bass.AP(tensor=src.tensor, offset=src.offset,
                               ap=[[0, 128]] + list(src.ap)))
        return t

    bo_bc = bcast(bo, "bo") if bo is not None else None
    lnw_bc = bcast(ln2w, "lnw") if general_ln else None
    lnb_bc = bcast(ln2b, "lnb") if general_ln else None
    b2_bc = bcast(b2, "b2") if b2 is not None else None

    wo_sb = const.tile([128, NCC, C], BF16, tag="wo")
    nc.sync.dma_start(out=wo_sb, in_=wo.rearrange("(cc p) c -> p cc c", p=128))
    ctT = const.tile([128, NCC, RPC], BF16, tag="catT")
    nc.sync.dma_start(out=ctT, in_=catT.rearrange("(cc p) t -> p cc t", p=128))

    xrp = ctx.enter_context(tc.tile_pool(name="xrp", bufs=2))
    x2p = ctx.enter_context(tc.tile_pool(name="x2p", bufs=NRB))
    hp = ctx.enter_context(tc.tile_pool(name="hp", bufs=2))
    h2Tp = ctx.enter_context(tc.tile_pool(name="h2Tp", bufs=1))
    stat = ctx.enter_context(tc.tile_pool(name="stat", bufs=8))
    w1p = ctx.enter_context(tc.tile_pool(name="w1p", bufs=4))
    w2p = ctx.enter_context(tc.tile_pool(name="w2p", bufs=4))
    ftp = ctx.enter_context(tc.tile_pool(name="ftp", bufs=NHID))
    tmp = ctx.enter_context(tc.tile_pool(name="tmp", bufs=3))
    osb = ctx.enter_context(tc.tile_pool(name="osb", bufs=2))

    x2_tiles = []
    h2T = h2Tp.tile([128, NCC, RPC], wdt, tag="h2T")
    with tc.tile_pool(name="psp", bufs=2, space="PSUM") as PSP, \
         tc.tile_pool(name="pst", bufs=2, space="PSUM") as PST:
        # ---- proj + residual + LN2 + transpose, pipelined per r ----
        for r in range(NRB):
            rsl = slice(r * 128, (r + 1) * 128)
            xt = xrp.tile([128, C], F32, tag="xr", name=f"xr_{r}")
            nc.sync.dma_start(out=xt, in_=xr[rsl, :])
            pps = PSP.tile([128, C], F32, tag="pp", name=f"pp_{r}")
            for cc in range(NCC):
                for co in range(2):
                    csl = slice(co * 512, (co + 1) * 512)
                    nc.tensor.matmul(pps[:, csl], ctT[:, cc, rsl],
                                     wo_sb[:, cc, csl],
                                     start=(cc == 0), stop=(cc == NCC - 1))
            x2t = x2p.tile([128, C], F32, tag="x2", name=f"x2_{r}")
            nc.vector.tensor_add(out=x2t, in0=pps, in1=xt)
            if bo_bc is not None:
                nc.vector.tensor_add(out=x2t, in0=x2t, in1=bo_bc)
            x2_tiles.append(x2t)
            # LN2 on this row block
            st = stat.tile([128, 2, 6], F32, tag="bn", name=f"bn_{r}")
            for k in range(2):
                nc.vector.bn_stats(out=st[:, k, :],
                                   in_=x2t[:, k * 512:(k + 1) * 512])
            mv = stat.tile([128, 2], F32, tag="mv", name=f"mv_{r}")
            nc.vector.bn_aggr(out=mv, in_=st)
            lnv = stat.tile([128, 1], F32, tag="lnv", name=f"lnv_{r}")
            nc.scalar.activation(out=lnv, in_=mv[:, 1:2], func=AF.Ln,
                                 bias=eps_t)
            rstd = stat.tile([128, 1], F32, tag="rstd", name=f"rstd_{r}")
            nc.scalar.activation(out=rstd, in_=lnv, func=AF.Exp, scale=-0.5)
            ht = hp.tile([128, C], BF16, tag="h", name=f"h_{r}")
            nc.vector.tensor_scalar(
                out=ht, in0=x2t, scalar1=mv[:, 0:1], scalar2=rstd,
                op0=ALU.subtract, op1=ALU.mult)
            if general_ln:
                nc.vector.tensor_mul(out=ht, in0=ht, in1=lnw_bc)
                nc.vector.tensor_add(out=ht, in0=ht, in1=lnb_bc)
            for g in range(2):
                pt = PST.tile([128, 4, 128], BF16, tag="tr4",
                              name=f"pt_{r}_{g}")
                for j in range(4):
                    cc = g * 4 + j
                    nc.tensor.transpose(pt[:, j, :],
                                        ht[:, cc * 128:(cc + 1) * 128], ident)
                nc.scalar.copy(out=h2T[:, g * 4:(g + 1) * 4, rsl], in_=pt)

    with tc.tile_pool(name="psf", bufs=2, space="PSUM") as PSF:
        # ---- phase 1: fT[h] = Lrelu(W1_h^T @ h2 + b1) ----
        f_tiles = []
        w1r = w1.rearrange("(cc p) (h q) -> p cc h q", p=128, q=128)
        for h in range(NHID):
            w1_sb = w1p.tile([128, NCC, 128], wdt, tag="w1",
                             name=f"w1_{h}")
            nc.sync.dma_start(out=w1_sb, in_=w1r[:, :, h, :])
            pf = PSF.tile([128, RPC], F32, tag="ft", name=f"pf_{h}")
            for cc in range(NCC):
                nc.tensor.matmul(pf, w1_sb[:, cc, :], h2T[:, cc, :],
                                 start=(cc == 0), stop=(cc == NCC - 1))
            ft = ftp.tile([128, RPC], wdt, tag="ft", name=f"ft_{h}")
            if b1_sb is not None:
                src = tmp.tile([128, RPC], F32, tag="pb", name=f"pb_{h}")
                nc.vector.tensor_scalar_add(out=src, in0=pf,
                                            scalar1=b1_sb[:, h:h + 1])
            else:
                src = pf
            # PReLU: ft = src + (alpha-1)*min(src, 0). (AF.Lrelu's alpha
            # operand is ignored by this lowering — it computes plain relu.)
            tneg = tmp.tile([128, RPC], F32, tag="tneg", name=f"tneg_{h}")
            nc.vector.tensor_scalar(
                out=tneg, in0=pf if b1_sb is None else src, scalar1=0.0,
                scalar2=alpha - 1.0, op0=ALU.min, op1=ALU.mult)
            nc.vector.tensor_add(out=ft, in0=src, in1=tneg)
            f_tiles.append(ft)

    # ---- phase 2: out = fT.T @ W2 (+b2) + x2 ----
    with tc.tile_pool(name="pso", bufs=NRB, space="PSUM") as PSO:
        pouts = [PSO.tile([128, C], F32, tag="out", name=f"pout{r}")
                 for r in range(NRB)]
        for h in range(NHID):
            w2_sb = w2p.tile([128, C], wdt, tag="w2", name=f"w2_{h}")
            nc.sync.dma_start(out=w2_sb, in_=w2[h * 128:(h + 1) * 128, :])
            for r in range(NRB):
                for co in range(2):
                    csl = slice(co * 512, (co + 1) * 512)
                    nc.tensor.matmul(pouts[r][:, csl],
                                     f_tiles[h][:, r * 128:(r + 1) * 128],
                                     w2_sb[:, csl],
                                     start=(h == 0), stop=(h == NHID - 1))
        for r in range(NRB):
            o_sb = osb.tile([128, C], F32, tag="o", name=f"o_{r}")
            nc.vector.tensor_add(out=o_sb, in0=pouts[r], in1=x2_tiles[r])
            if b2_bc is not None:
                nc.vector.tensor_add(out=o_sb, in0=o_sb, in1=b2_bc)
            nc.sync.dma_start(out=out[r * 128:(r + 1) * 128, :], in_=o_sb)


def _build_ffn(general_ln: bool, has_bo: bool, has_b1: bool, has_b2: bool,
               alpha: float, repeat: int = 1, wdt=F32R):
    nc = bacc.Bacc("TRN2", target_bir_lowering=False, debug=False)
    xr = nc.dram_tensor("xr", [RPC, C], F32, kind="ExternalInput").ap()
    catT = nc.dram_tensor("catT", [C, RPC], BF16, kind="ExternalInput").ap()
    wo = nc.dram_tensor("wo", [C, C], BF16, kind="ExternalInput").ap()
    w1 = nc.dram_tensor("w1", [C, 4 * C], wdt, kind="ExternalInput").ap()
    w2 = nc.dram_tensor("w2", [4 * C, C], wdt, kind="ExternalInput").ap()
    bo = b1 = ln2w = ln2b = b2 = None
    if has_bo:
        bo = nc.dram_tensor("bo", [C], F32, kind="ExternalInput").ap()
    if has_b1:
        b1 = nc.dram_tensor("b1", [4 * C], F32, kind="ExternalInput").ap()
    if general_ln:
        ln2w = nc.dram_tensor("ln2w", [C], F32, kind="ExternalInput").ap()
        ln2b = nc.dram_tensor("ln2b", [C], F32, kind="ExternalInput").ap()
    if has_b2:
        b2 = nc.dram_tensor("b2", [C], F32, kind="ExternalInput").ap()
    out = nc.dram_tensor("out", [RPC, C], F32, kind="ExternalOutput").ap()
    with tile.TileContext(nc) as tc:
        for _ in range(repeat):
            with ExitStack() as ctx:
                _ffn_body(ctx, tc, xr, catT, wo, w1, w2, bo, b1,
                          ln2w, ln2b, b2, alpha, out, wdt=wdt)
    nc.compile()
    return nc


# --------------------------------------------------------------------------
# host orchestration
# --------------------------------------------------------------------------

_NC_CACHE = {}


def _get_attn_nc(general_ln):
    key = ("attn", general_ln)
    if key not in _NC_CACHE:
        _NC_CACHE[key] = _build_attn(general_ln)
    return _NC_CACHE[key]


def _get_ffn_nc(general_ln, has_bo, has_b1, has_b2, alpha, wdt=None):
    wdt = FFN_WDT if wdt is None else wdt
    key = ("ffn", general_ln, has_bo, has_b1, has_b2, float(alpha), wdt)
    if key not in _NC_CACHE:
        _NC_CACHE[key] = _build_ffn(general_ln, has_bo, has_b1, has_b2,
                                    float(alpha), wdt=wdt)
    return _NC_CACHE[key]


def _w_np(a):
    if FFN_WDT == BF16:
        import ml_dtypes
        return np.ascontiguousarray(a.astype(ml_dtypes.bfloat16))
    return a


def _bf(a):
    import ml_dtypes
    return np.ascontiguousarray(np.asarray(a).astype(ml_dtypes.bfloat16))


def attn_in_maps(x_flat, Wq, Wk, Wv, trivial, ln1_w, ln1_b):
    x_bf = _bf(x_flat)
    in_maps = []
    for c in range(NCORES):
        h0 = 2 * c
        m = {
            "x": x_bf,
            "wq": _bf(np.concatenate([Wq[h0], Wq[h0 + 1]], axis=1)),
            "wk": _bf(np.concatenate([Wk[h0], Wk[h0 + 1]], axis=1)),
            "wv": _bf(np.concatenate([Wv[h0], Wv[h0 + 1]], axis=1)),
        }
        if not trivial:
            m["lnw"] = ln1_w
            m["lnb"] = ln1_b
        in_maps.append(m)
    return in_maps


def run_attn(x_flat, Wq, Wk, Wv, ln1_w, ln1_b):
    """Returns catT [C, B*T] bf16: transposed per-head attention outputs."""
    trivial = bool(np.all(ln1_w == 1.0) and np.all(ln1_b == 0.0))
    nc = _get_attn_nc(not trivial)
    in_maps = attn_in_maps(x_flat, Wq, Wk, Wv, trivial, ln1_w, ln1_b)
    res = run_bass_kernel_spmd(nc, in_maps, list(range(NCORES)), trace=False)
    return np.concatenate(
        [res.results[c]["catT"] for c in range(NCORES)], axis=0)


def ffn_in_maps(x_flat, catT_all, Wo, bo, W1, b1, W2, b2, ln2_w, ln2_b,
                flags):
    trivial, has_bo, has_b1, has_b2 = flags
    wo_np = _bf(Wo)
    w1_np, w2_np = _w_np(W1), _w_np(W2)
    in_maps = []
    for c in range(NCORES):
        sl = slice(RPC * c, RPC * (c + 1))
        m = {
            "xr": np.ascontiguousarray(x_flat[sl]),
            "catT": np.ascontiguousarray(catT_all[:, sl]),
            "wo": wo_np,
            "w1": w1_np,
            "w2": w2_np,
        }
        if has_bo:
            m["bo"] = bo
        if has_b1:
            m["b1"] = b1
        if not trivial:
            m["ln2w"] = ln2_w
            m["ln2b"] = ln2_b
        if has_b2:
            m["b2"] = b2
        in_maps.append(m)
    return in_maps


def run_ffn(x_flat, catT_all, Wo, bo, W1, b1, W2, b2, ln2_w, ln2_b, alpha):
    trivial = bool(np.all(ln2_w == 1.0) and np.all(ln2_b == 0.0))
    has_bo = bool(np.any(bo != 0.0))
    has_b1 = bool(np.any(b1 != 0.0))
    has_b2 = bool(np.any(b2 != 0.0))
    nc = _get_ffn_nc(not trivial, has_bo, has_b1, has_b2, alpha)
    flags = (trivial, has_bo, has_b1, has_b2)
    in_maps = ffn_in_maps(x_flat, catT_all, Wo, bo, W1, b1, W2, b2,
                          ln2_w, ln2_b, flags)
    res = run_bass_kernel_spmd(nc, in_maps, list(range(NCORES)), trace=False)
    return np.concatenate(
        [res.results[c]["out"] for c in range(NCORES)], axis=0)


def kernel(x, ln1_w, ln1_b, Wk, Wq, Wv, Wo, bo, ln2_w, ln2_b, W1, b1,
           prelu_a, W2, b2):
    x = np.asarray(x, np.float32)
    x_flat = np.ascontiguousarray(x.reshape(B * T, C))
    Wq = np.asarray(Wq, np.float32)
    Wk = np.asarray(Wk, np.float32)
    Wv = np.asarray(Wv, np.float32)
    Wo = np.asarray(Wo, np.float32)
    alpha = float(np.asarray(prelu_a))

    catT_all = run_attn(x_flat, Wq, Wk, Wv,
                        np.asarray(ln1_w, np.float32),
                        np.asarray(ln1_b, np.float32))
    out = run_ffn(x_flat, catT_all, Wo, np.asarray(bo, np.float32),
                  np.asarray(W1, np.float32), np.asarray(b1, np.float32),
                  np.asarray(W2, np.float32), np.asarray(b2, np.float32),
                  np.asarray(ln2_w, np.float32),
                  np.asarray(ln2_b, np.float32), alpha)
    return out.reshape(B, T, C).astype(np.float32)



# revision 50
# speedup vs baseline: 1.2475x; 1.0584x over previous
"""Trainium2 Bass kernel for a dense pre-LN transformer block.

B=4, T=1024, C=1024, H=16 heads (head_size 64).

Distribution over the 8 NeuronCores (two SPMD launches, host-side
reduction between them):

  Launch A (attention, head-parallel): every core runs the identical
  program on all 4 batches but with its own pair of heads (weight
  slices are per-core input data). Each core computes LN1, its 2 heads'
  q/k/v + causal attention, and the partial Wo projection of those
  heads for the whole [B*T, C] output (written f32 from PSUM).
  NOTE the reference computes scores as k @ q^T (roles of q/k swapped
  vs standard attention) — handled by using k rows as the "queries".

  Host: x2 = x + sum_c projpart_c + bo.

  Launch B (FFN, row-parallel): core c runs LN2 + W1/PReLU/W2 + residual
  on rows [512c, 512(c+1)) of x2.

Matmuls run in bf16 (1 cyc/row on the PE regardless of output width;
fp32r drops to 1/4 rate for outputs narrower than 256). LN normalizes
run on the DVE — GpSimd is ~30x slower and was the original bottleneck.
"""

from contextlib import ExitStack

import numpy as np

import concourse.bass as bass
import concourse.tile as tile
from concourse import bacc, mybir
from concourse.bass_utils import run_bass_kernel_spmd
from concourse.masks import make_identity, make_causal_mask

F32 = mybir.dt.float32
F32R = mybir.dt.float32r
BF16 = mybir.dt.bfloat16
# FFN W1/W2/fT/h2T dtype: BF16 halves the dominant 32MB weight stream
# (rel-err impact validated on HW before adoption)
FFN_WDT = BF16
AF = mybir.ActivationFunctionType
ALU = mybir.AluOpType

B, T, C, H, HS = 4, 1024, 1024, 16, 64
NCORES = 8
EPS = 1e-5
SCALE = float(C) ** -0.5  # 1/32, folded into the softmax exp
NEG = -1e30

NTB = T // 128   # 8 token blocks per batch
NCC = C // 128   # 8 channel chunks


# --------------------------------------------------------------------------
# kernel A: attention, 2 heads per core, all batches
# --------------------------------------------------------------------------

def _attn_body(ctx, tc, x, wq, wk, wv, lnw, lnb, catoutT):
    """Transposed-scores attention: scoresT[s,t] with s on partitions.

    softmax denominator comes from an appended ones-column in v (av psum
    column 64), normalization is a per-partition scale on the av output,
    so no wei transposes are needed; only [t,d]->[d,t] cat transposes.

    All matmul operands are bf16 (1 cyc/row on the PE even for <256-wide
    outputs, where fp32r drops to 1/4 rate); LN normalize runs on the DVE
    (GpSimd is ~30x slower and was the launch bottleneck).

    Each matmul pays ~75ns of weight-load overhead, so matmuls are batched
    to the widest moving dim possible: scores in <=512-wide chunks, v via
    a [d,t] projection (512-wide) + per-block PE transposes.

    The per-head outputs are written TRANSPOSED (catT [128 d, B*T]) so
    launch B can feed its Wo projection without any transposes.
    """
    nc = tc.nc

    const = ctx.enter_context(tc.tile_pool(name="const", bufs=1))
    scratch = const.tile([128, 128], F32)
    make_identity(nc, scratch)
    ident = const.tile([128, 128], BF16)
    nc.vector.tensor_copy(out=ident, in_=scratch)
    eps_t = const.tile([128, 1], F32)
    nc.vector.memset(eps_t, EPS)

    wq_sb = const.tile([128, NCC, 128], BF16, tag="wq")
    wk_sb = const.tile([128, NCC, 128], BF16, tag="wk")
    wv_sb = const.tile([128, NCC, 128], BF16, tag="wv")
    nc.sync.dma_start(out=wq_sb, in_=wq.rearrange("(cc p) d -> p cc d", p=128))
    nc.sync.dma_start(out=wk_sb, in_=wk.rearrange("(cc p) d -> p cc d", p=128))
    nc.sync.dma_start(out=wv_sb, in_=wv.rearrange("(cc p) d -> p cc d", p=128))
    general_ln = lnw is not None
    if general_ln:
        lnw_bc = const.tile([128, C], F32, tag="lnw")
        lnb_bc = const.tile([128, C], F32, tag="lnb")
        nc.sync.dma_start(
            out=lnw_bc,
            in_=bass.AP(tensor=lnw.tensor, offset=lnw.offset,
                        ap=[[0, 128]] + list(lnw.ap)))
        nc.sync.dma_start(
            out=lnb_bc,
            in_=bass.AP(tensor=lnb.tensor, offset=lnb.offset,
                        ap=[[0, 128]] + list(lnb.ap)))

    xp = ctx.enter_context(tc.tile_pool(name="xp", bufs=9))
    hp = ctx.enter_context(tc.tile_pool(name="hp", bufs=9))
    hTp = ctx.enter_context(tc.tile_pool(name="hTp", bufs=1))
    stat = ctx.enter_context(tc.tile_pool(name="stat", bufs=4))
    qkp = ctx.enter_context(tc.tile_pool(name="qkp", bufs=2))
    vp = ctx.enter_context(tc.tile_pool(name="vp", bufs=2))
    epl = ctx.enter_context(tc.tile_pool(name="epl", bufs=2))
    ctkp = ctx.enter_context(tc.tile_pool(name="ctkp", bufs=10))
    catBp = ctx.enter_context(tc.tile_pool(name="catBp", bufs=2))

    # PSUM banks: mm 2x[128,512]=2, score 2x[128,1024]=4, tr4 2x[128,512]=2
    PSM = ctx.enter_context(tc.tile_pool(name="psm", bufs=2, space="PSUM"))
    PSS = ctx.enter_context(tc.tile_pool(name="pss", bufs=2, space="PSUM"))
    PST = ctx.enter_context(tc.tile_pool(name="pst", bufs=2, space="PSUM"))

    for b in range(B):
        # ---- LN1: one Ln + one Exp per batch (ACT table switches cost
        # ~1.3us each, so Ln/Exp must not alternate per group) ----
        h_tiles = []
        mvs = stat.tile([128, NTB, 2], F32, tag="mvs", name=f"mvs_{b}")
        rstd = stat.tile([128, NTB], F32, tag="rstd", name=f"rstd_{b}")
        lnv = stat.tile([128, NTB], F32, tag="lnv", name=f"lnv_{b}")
        xts = []
        for i in range(NTB):
            r0 = (b * NTB + i) * 128
            xt = xp.tile([128, C], BF16, tag="x", name=f"x_{b}_{i}")
            nc.sync.dma_start(out=xt, in_=x[r0:r0 + 128, :])
            st = stat.tile([128, 2, 6], F32, tag="bn", name=f"bn_{b}_{i}")
            for k in range(2):
                nc.vector.bn_stats(out=st[:, k, :],
                                   in_=xt[:, k * 512:(k + 1) * 512])
            nc.vector.bn_aggr(out=mvs[:, i, :], in_=st)
            xts.append(xt)
        nc.scalar.activation(out=lnv, in_=mvs[:, :, 1], func=AF.Ln,
                             bias=eps_t)
        nc.scalar.activation(out=rstd, in_=lnv, func=AF.Exp, scale=-0.5)
        for i in range(NTB):
            ht = hp.tile([128, C], BF16, tag="h", name=f"h_{b}_{i}")
            nc.vector.tensor_scalar(
                out=ht, in0=xts[i], scalar1=mvs[:, i, 0:1],
                scalar2=rstd[:, i:i + 1], op0=ALU.subtract, op1=ALU.mult)
            if general_ln:
                nc.vector.tensor_mul(out=ht, in0=ht, in1=lnw_bc)
                nc.vector.tensor_add(out=ht, in0=ht, in1=lnb_bc)
            h_tiles.append(ht)

        # ---- transpose h -> hT, grouped 4 blocks per psum/copy ----
        hT = hTp.tile([128, NCC, T], BF16, tag="hT")
        for cc in range(NCC):
            for g in range(2):
                pt = PST.tile([128, 512], BF16, tag="tr4",
                              name=f"pt_{b}_{cc}_{g}")
                for j in range(4):
                    i = g * 4 + j
                    nc.tensor.transpose(
                        pt[:, j * 128:(j + 1) * 128],
                        h_tiles[i][:, cc * 128:(cc + 1) * 128], ident)
                eng = nc.vector if (cc + g) % 2 else nc.scalar
                if eng is nc.scalar:
                    nc.scalar.copy(
                        out=hT[:, cc, g * 512:(g + 1) * 512], in_=pt)
                else:
                    nc.vector.tensor_copy(
                        out=hT[:, cc, g * 512:(g + 1) * 512], in_=pt)

        # ---- qkv (2 heads packed: d2 = 128) ----
        qT2 = qkp.tile([128, T], BF16, tag="qT", name=f"qT_{b}")
        kT2 = qkp.tile([128, T], BF16, tag="kT", name=f"kT_{b}")
        for tch in range(T // 512):
            tsl = slice(tch * 512, (tch + 1) * 512)
            pq = PSM.tile([128, 512], F32, tag="mm", name=f"pq_{b}_{tch}")
            for cc in range(NCC):
                nc.tensor.matmul(pq, wq_sb[:, cc, :], hT[:, cc, tsl],
                                 start=(cc == 0), stop=(cc == NCC - 1))
            nc.scalar.copy(out=qT2[:, tsl], in_=pq)
            pk = PSM.tile([128, 512], F32, tag="mm", name=f"pk_{b}_{tch}")
            for cc in range(NCC):
                nc.tensor.matmul(pk, wk_sb[:, cc, :], hT[:, cc, tsl],
                                 start=(cc == 0), stop=(cc == NCC - 1))
            nc.scalar.copy(out=kT2[:, tsl], in_=pk)
        # vT: [d2, t] via 512-wide streams (same form as q/k)
        vT2 = qkp.tile([128, T], BF16, tag="vT", name=f"vT_{b}")
        for tch in range(T // 512):
            tsl = slice(tch * 512, (tch + 1) * 512)
            pv = PSM.tile([128, 512], F32, tag="mm", name=f"pvT_{b}_{tch}")
            for cc in range(NCC):
                nc.tensor.matmul(pv, wv_sb[:, cc, :], hT[:, cc, tsl],
                                 start=(cc == 0), stop=(cc == NCC - 1))
            nc.scalar.copy(out=vT2[:, tsl], in_=pv)
        # v2: [t_part, sc, 130]: per head 65 cols (64 v + ones), built by
        # transposing vT back to [t, d] in groups of 4 blocks per psum tile
        v2 = vp.tile([128, NTB, 132], BF16, tag="v2", name=f"v2_{b}")
        nc.vector.memset(v2, 0.0)
        nc.vector.memset(v2[:, :, 64:65], 1.0)
        nc.vector.memset(v2[:, :, 130:131], 1.0)
        for g in range(2):
            ptv = PST.tile([128, 512], BF16, tag="tr4", name=f"ptv_{b}_{g}")
            for j in range(4):
                i = g * 4 + j
                nc.tensor.transpose(
                    ptv[:, j * 128:(j + 1) * 128],
                    vT2[:, i * 128:(i + 1) * 128], ident)
            for j in range(4):
                i = g * 4 + j
                nc.vector.tensor_copy(out=v2[:, i, 0:64],
                                      in_=ptv[:, j * 128:j * 128 + 64])
                nc.vector.tensor_copy(out=v2[:, i, 66:130],
                                      in_=ptv[:, j * 128 + 64:(j + 1) * 128])

        # ---- attention ----
        cat_toks = [ctkp.tile([128, 128], BF16, tag="ctk",
                              name=f"ctk_{b}_{i}") for i in range(NTB)]
        for h in range(2):
            hsl = slice(h * 64, (h + 1) * 64)
            # scoresT + exp, one psum + one exp per s-chunk
            eps_list = []
            for sc in range(NTB):
                W = (NTB - sc) * 128  # t columns: blocks sc..7
                pss = PSS.tile([128, W], F32, tag="score",
                               name=f"pss_{b}_{h}_{sc}")
                # one matmul per 512-aligned chunk (t cols are contiguous
                # in kT2); per-128-block matmuls waste ~75ns each on
                # weight (re)loads of the same qT block
                for n0 in range(0, W, 512):
                    n1 = min(n0 + 512, W)
                    nc.tensor.matmul(
                        pss[:, n0:n1],
                        qT2[hsl, sc * 128:(sc + 1) * 128],
                        kT2[hsl, sc * 128 + n0:sc * 128 + n1],
                        start=True, stop=True)
                e_sc = epl.tile([128, W], BF16, tag=f"e{sc}",
                                name=f"e_{b}_{h}_{sc}")
                # one exp op per PSUM bank (bank-crossing ACT reads are
                # suspect for the NRT_EXEC_UNIT_UNRECOVERABLE wedge)
                n0 = 0
                while n0 < W:
                    n1 = min(n0 + 512, W)
                    nc.scalar.activation(out=e_sc[:, n0:n1],
                                         in_=pss[:, n0:n1], func=AF.Exp,
                                         scale=SCALE)
                    n0 = n1
                # causal mask on the diagonal block: zero out s>t entries
                # post-exp (runs on the otherwise-idle gpsimd engine)
                nc.gpsimd.affine_select(
                    out=e_sc[:, 0:128], in_=e_sc[:, 0:128],
                    compare_op=ALU.is_ge, fill=0.0, base=0,
                    pattern=[[1, 128]], channel_multiplier=-1)
                eps_list.append(e_sc)
            # av + normalize into cat_tok
            for i in range(NTB):
                po = PSM.tile([128, 66], F32, tag="mm",
                              name=f"po_{b}_{h}_{i}")
                for sc in range(i + 1):
                    j = i - sc
                    nc.tensor.matmul(
                        po, eps_list[sc][:, j * 128:(j + 1) * 128],
                        v2[:, sc, h * 66:(h + 1) * 66],
                        start=(sc == 0), stop=(sc == i))
                rec = stat.tile([128, 1], F32, tag="rec",
                                name=f"rec_{b}_{h}_{i}")
                nc.vector.reciprocal(out=rec, in_=po[:, 64:65])
                nc.vector.tensor_scalar_mul(
                    out=cat_toks[i][:, hsl], in0=po[:, 0:64], scalar1=rec)

        # ---- transpose cat -> [d, t] and write catT to DRAM ----
        # (launch B consumes catT directly for the Wo projection, so it
        # does no transposes and starts its matmuls immediately)
        for g in range(2):
            ptc = PST.tile([128, 512], BF16, tag="tr4", name=f"ptc_{b}_{g}")
            for j in range(4):
                nc.tensor.transpose(
                    ptc[:, j * 128:(j + 1) * 128], cat_toks[g * 4 + j], ident)
            catB = catBp.tile([128, 512], BF16, tag="catB",
                              name=f"catB_{b}_{g}")
            nc.vector.tensor_copy(out=catB, in_=ptc)
            c0 = b * T + g * 512
            nc.sync.dma_start(out=catoutT[:, c0:c0 + 512], in_=catB)


def _build_attn(general_ln: bool, repeat: int = 1):
    nc = bacc.Bacc("TRN2", target_bir_lowering=False, debug=False)
    x = nc.dram_tensor("x", [B * T, C], BF16, kind="ExternalInput").ap()
    wq = nc.dram_tensor("wq", [C, 128], BF16, kind="ExternalInput").ap()
    wk = nc.dram_tensor("wk", [C, 128], BF16, kind="ExternalInput").ap()
    wv = nc.dram_tensor("wv", [C, 128], BF16, kind="ExternalInput").ap()
    lnw = lnb = None
    if general_ln:
        lnw = nc.dram_tensor("lnw", [C], F32, kind="ExternalInput").ap()
        lnb = nc.dram_tensor("lnb", [C], F32, kind="ExternalInput").ap()
    catoutT = nc.dram_tensor("catT", [128, B * T], BF16,
                             kind="ExternalOutput").ap()
    with tile.TileContext(nc) as tc:
        for _ in range(repeat):
            with ExitStack() as ctx:
                _attn_body(ctx, tc, x, wq, wk, wv, lnw, lnb, catoutT)
    nc.compile()
    return nc


# --------------------------------------------------------------------------
# kernel B: FFN, 512 rows per core
# --------------------------------------------------------------------------

RPC = (B * T) // NCORES  # 512 rows per core
NRB = RPC // 128         # 4 row blocks
NHID = 4 * C // 128      # 32 hidden chunks


def _ffn_body(ctx, tc, xr, catT, wo, w1, w2, bo, b1, ln2w, ln2b, b2,
              alpha, out, wdt=F32R):
    """Per-core rows: proj = catT.T @ Wo (+bo); x2 = x + proj; LN2 + FFN.

    catT arrives pre-transposed from launch A, so the projection is pure
    matmul and pipelines per row-block with LN2/h2T. PReLU is a single
    scalar-engine Lrelu op. wdt: dtype for W1/W2/fT/h2T.
    """
    nc = tc.nc
    general_ln = ln2w is not None

    const = ctx.enter_context(tc.tile_pool(name="const", bufs=1))
    scratch = const.tile([128, 128], F32)
    make_identity(nc, scratch)
    ident = const.tile([128, 128], BF16)
    nc.vector.tensor_copy(out=ident, in_=scratch)
    eps_t = const.tile([128, 1], F32)
    nc.vector.memset(eps_t, EPS)
    b1_sb = None
    if b1 is not None:
        b1_sb = const.tile([128, NHID], F32, tag="b1")
        nc.sync.dma_start(out=b1_sb, in_=b1.rearrange("(h p) -> p h", p=128))

    def bcast(src, tag):
        t = const.tile([128, C], F32, tag=tag, name=tag)
        nc.sync.dma_start(
            out=t, in_=bass.AP(tensor=src.tensor, offset=src.offset,
                               ap=[[0, 128]] + list(src.ap)))
        return t

    bo_bc = bcast(bo, "bo") if bo is not None else None
    lnw_bc = bcast(ln2w, "lnw") if general_ln else None
    lnb_bc = bcast(ln2b, "lnb") if general_ln else None
    b2_bc = bcast(b2, "b2") if b2 is not None else None

    wo_sb = const.tile([128, NCC, C], BF16, tag="wo")
    nc.sync.dma_start(out=wo_sb, in_=wo.rearrange("(cc p) c -> p cc c", p=128))
    ctT = const.tile([128, NCC, RPC], BF16, tag="catT")
    nc.sync.dma_start(out=ctT, in_=catT.rearrange("(cc p) t -> p cc t", p=128))

    xrp = ctx.enter_context(tc.tile_pool(name="xrp", bufs=2))
    x2p = ctx.enter_context(tc.tile_pool(name="x2p", bufs=NRB))
    hp = ctx.enter_context(tc.tile_pool(name="hp", bufs=2))
    h2Tp = ctx.enter_context(tc.tile_pool(name="h2Tp", bufs=1))
    stat = ctx.enter_context(tc.tile_pool(name="stat", bufs=8))
    w1p = ctx.enter_context(tc.tile_pool(name="w1p", bufs=4))
    w2p = ctx.enter_context(tc.tile_pool(name="w2p", bufs=4))
    ftp = ctx.enter_context(tc.tile_pool(name="ftp", bufs=NHID))
    tmp = ctx.enter_context(tc.tile_pool(name="tmp", bufs=3))
    osb = ctx.enter_context(tc.tile_pool(name="osb", bufs=2))

    x2_tiles = []
    h2T = h2Tp.tile([128, NCC, RPC], wdt, tag="h2T")
    with tc.tile_pool(name="psp", bufs=2, space="PSUM") as PSP, \
         tc.tile_pool(name="pst", bufs=2, space="PSUM") as PST:
        # ---- proj + residual + LN2 + transpose, pipelined per r ----
        for r in range(NRB):
            rsl = slice(r * 128, (r + 1) * 128)
            xt = xrp.tile([128, C], F32, tag="xr", name=f"xr_{r}")
            nc.sync.dma_start(out=xt, in_=xr[rsl, :])
            pps = PSP.tile([128, C], F32, tag="pp", name=f"pp_{r}")
            for cc in range(NCC):
                for co in range(2):
                    csl = slice(co * 512, (co + 1) * 512)
                    nc.tensor.matmul(pps[:, csl], ctT[:, cc, rsl],
                                     wo_sb[:, cc, csl],
                                     start=(cc == 0), stop=(cc == NCC - 1))
            x2t = x2p.tile([128, C], F32, tag="x2", name=f"x2_{r}")
            nc.vector.tensor_add(out=x2t, in0=pps, in1=xt)
            if bo_bc is not None:
                nc.vector.tensor_add(out=x2t, in0=x2t, in1=bo_bc)
            x2_tiles.append(x2t)
            # LN2 on this row block
            st = stat.tile([128, 2, 6], F32, tag="bn", name=f"bn_{r}")
            for k in range(2):
                nc.vector.bn_stats(out=st[:, k, :],
                                   in_=x2t[:, k * 512:(k + 1) * 512])
            mv = stat.tile([128, 2], F32, tag="mv", name=f"mv_{r}")
            nc.vector.bn_aggr(out=mv, in_=st)
            lnv = stat.tile([128, 1], F32, tag="lnv", name=f"lnv_{r}")
            nc.scalar.activation(out=lnv, in_=mv[:, 1:2], func=AF.Ln,
                                 bias=eps_t)
            rstd = stat.tile([128, 1], F32, tag="rstd", name=f"rstd_{r}")
            nc.scalar.activation(out=rstd, in_=lnv, func=AF.Exp, scale=-0.5)
            ht = hp.tile([128, C], BF16, tag="h", name=f"h_{r}")
            nc.vector.tensor_scalar(
                out=ht, in0=x2t, scalar1=mv[:, 0:1], scalar2=rstd,
                op0=ALU.subtract, op1=ALU.mult)
            if general_ln:
                nc.vector.tensor_mul(out=ht, in0=ht, in1=lnw_bc)
                nc.vector.tensor_add(out=ht, in0=ht, in1=lnb_bc)
            for g in range(2):
                pt = PST.tile([128, 4, 128], BF16, tag="tr4",
                              name=f"pt_{r}_{g}")
                for j in range(4):
                    cc = g * 4 + j
                    nc.tensor.transpose(pt[:, j, :],
                                        ht[:, cc * 128:(cc + 1) * 128], ident)
                nc.scalar.copy(out=h2T[:, g * 4:(g + 1) * 4, rsl], in_=pt)

    with tc.tile_pool(name="psf", bufs=2, space="PSUM") as PSF:
        # ---- phase 1: fT[h] = prelu(W1_h^T @ h2 + b1) ----
        # w1 arrives host-transposed as [NHID/2, 128, 2, NCC, 128] so each
        # DMA reads contiguous 4KB partition lines (2 h-chunks per DMA)
        f_tiles = []
        for h2i in range(NHID // 2):
            w1_sb = w1p.tile([128, 2, NCC, 128], wdt, tag="w1",
                             name=f"w1_{h2i}")
            nc.sync.dma_start(out=w1_sb, in_=w1[h2i])
            for s in range(2):
                h = 2 * h2i + s
                pf = PSF.tile([128, RPC], F32, tag="ft", name=f"pf_{h}")
                for cc in range(NCC):
                    nc.tensor.matmul(pf, w1_sb[:, s, cc, :], h2T[:, cc, :],
                                     start=(cc == 0), stop=(cc == NCC - 1))
                ft = ftp.tile([128, RPC], wdt, tag="ft", name=f"ft_{h}")
                if b1_sb is not None:
                    src = tmp.tile([128, RPC], F32, tag="pb", name=f"pb_{h}")
                    nc.vector.tensor_scalar_add(out=src, in0=pf,
                                                scalar1=b1_sb[:, h:h + 1])
                else:
                    src = pf
                # PReLU: ft = src + (alpha-1)*min(src, 0). (AF.Lrelu's
                # alpha operand is ignored by this lowering — plain relu.)
                tneg = tmp.tile([128, RPC], F32, tag="tneg", name=f"tneg_{h}")
                nc.vector.tensor_scalar(
                    out=tneg, in0=src, scalar1=0.0,
                    scalar2=alpha - 1.0, op0=ALU.min, op1=ALU.mult)
                nc.vector.tensor_add(out=ft, in0=src, in1=tneg)
                f_tiles.append(ft)

    # ---- phase 2: out = fT.T @ W2 (+b2) + x2 ----
    with tc.tile_pool(name="pso", bufs=NRB, space="PSUM") as PSO:
        pouts = [PSO.tile([128, C], F32, tag="out", name=f"pout{r}")
                 for r in range(NRB)]
        for h2i in range(NHID // 2):
            w2_sb = w2p.tile([128, 2, C], wdt, tag="w2", name=f"w2_{h2i}")
            nc.sync.dma_start(
                out=w2_sb,
                in_=w2[h2i * 256:(h2i + 1) * 256, :].rearrange(
                    "(s p) c -> p s c", p=128))
            for s in range(2):
                h = 2 * h2i + s
                for r in range(NRB):
                    for co in range(2):
                        csl = slice(co * 512, (co + 1) * 512)
                        nc.tensor.matmul(pouts[r][:, csl],
                                         f_tiles[h][:, r * 128:(r + 1) * 128],
                                         w2_sb[:, s, csl],
                                         start=(h == 0), stop=(h == NHID - 1))
        for r in range(NRB):
            o_sb = osb.tile([128, C], F32, tag="o", name=f"o_{r}")
            nc.vector.tensor_add(out=o_sb, in0=pouts[r], in1=x2_tiles[r])
            if b2_bc is not None:
                nc.vector.tensor_add(out=o_sb, in0=o_sb, in1=b2_bc)
            nc.sync.dma_start(out=out[r * 128:(r + 1) * 128, :], in_=o_sb)


def _build_ffn(general_ln: bool, has_bo: bool, has_b1: bool, has_b2: bool,
               alpha: float, repeat: int = 1, wdt=F32R):
    nc = bacc.Bacc("TRN2", target_bir_lowering=False, debug=False)
    xr = nc.dram_tensor("xr", [RPC, C], F32, kind="ExternalInput").ap()
    catT = nc.dram_tensor("catT", [C, RPC], BF16, kind="ExternalInput").ap()
    wo = nc.dram_tensor("wo", [C, C], BF16, kind="ExternalInput").ap()
    # w1 is host-pretransposed to [h-pair, partition, s, cc, q] so each
    # per-pair DMA reads one contiguous 4KB line per partition
    w1 = nc.dram_tensor("w1", [NHID // 2, 128, 2, NCC, 128], wdt,
                        kind="ExternalInput").ap()
    w2 = nc.dram_tensor("w2", [4 * C, C], wdt, kind="ExternalInput").ap()
    bo = b1 = ln2w = ln2b = b2 = None
    if has_bo:
        bo = nc.dram_tensor("bo", [C], F32, kind="ExternalInput").ap()
    if has_b1:
        b1 = nc.dram_tensor("b1", [4 * C], F32, kind="ExternalInput").ap()
    if general_ln:
        ln2w = nc.dram_tensor("ln2w", [C], F32, kind="ExternalInput").ap()
        ln2b = nc.dram_tensor("ln2b", [C], F32, kind="ExternalInput").ap()
    if has_b2:
        b2 = nc.dram_tensor("b2", [C], F32, kind="ExternalInput").ap()
    out = nc.dram_tensor("out", [RPC, C], F32, kind="ExternalOutput").ap()
    with tile.TileContext(nc) as tc:
        for _ in range(repeat):
            with ExitStack() as ctx:
                _ffn_body(ctx, tc, xr, catT, wo, w1, w2, bo, b1,
                          ln2w, ln2b, b2, alpha, out, wdt=wdt)
    nc.compile()
    return nc


# --------------------------------------------------------------------------
# host orchestration
# --------------------------------------------------------------------------

_NC_CACHE = {}


def _get_attn_nc(general_ln):
    key = ("attn", general_ln)
    if key not in _NC_CACHE:
        _NC_CACHE[key] = _build_attn(general_ln)
    return _NC_CACHE[key]


def _get_ffn_nc(general_ln, has_bo, has_b1, has_b2, alpha, wdt=None):
    wdt = FFN_WDT if wdt is None else wdt
    key = ("ffn", general_ln, has_bo, has_b1, has_b2, float(alpha), wdt)
    if key not in _NC_CACHE:
        _NC_CACHE[key] = _build_ffn(general_ln, has_bo, has_b1, has_b2,
                                    float(alpha), wdt=wdt)
    return _NC_CACHE[key]


def _w_np(a):
    if FFN_WDT == BF16:
        import ml_dtypes
        return np.ascontiguousarray(a.astype(ml_dtypes.bfloat16))
    return a


def _bf(a):
    import ml_dtypes
    return np.ascontiguousarray(np.asarray(a).astype(ml_dtypes.bfloat16))


def attn_in_maps(x_flat, Wq, Wk, Wv, trivial, ln1_w, ln1_b):
    x_bf = _bf(x_flat)
    in_maps = []
    for c in range(NCORES):
        h0 = 2 * c
        m = {
            "x": x_bf,
            "wq": _bf(np.concatenate([Wq[h0], Wq[h0 + 1]], axis=1)),
            "wk": _bf(np.concatenate([Wk[h0], Wk[h0 + 1]], axis=1)),
            "wv": _bf(np.concatenate([Wv[h0], Wv[h0 + 1]], axis=1)),
        }
        if not trivial:
            m["lnw"] = ln1_w
            m["lnb"] = ln1_b
        in_maps.append(m)
    return in_maps


def run_attn(x_flat, Wq, Wk, Wv, ln1_w, ln1_b):
    """Returns catT [C, B*T] bf16: transposed per-head attention outputs."""
    trivial = bool(np.all(ln1_w == 1.0) and np.all(ln1_b == 0.0))
    nc = _get_attn_nc(not trivial)
    in_maps = attn_in_maps(x_flat, Wq, Wk, Wv, trivial, ln1_w, ln1_b)
    res = run_bass_kernel_spmd(nc, in_maps, list(range(NCORES)), trace=False)
    return np.concatenate(
        [res.results[c]["catT"] for c in range(NCORES)], axis=0)


def ffn_in_maps(x_flat, catT_all, Wo, bo, W1, b1, W2, b2, ln2_w, ln2_b,
                flags):
    trivial, has_bo, has_b1, has_b2 = flags
    wo_np = _bf(Wo)
    # pre-transpose W1 to [h-pair, p, s, cc, q] (see _build_ffn)
    w1_np = _w_np(np.ascontiguousarray(
        W1.reshape(NCC, 128, NHID // 2, 2, 128).transpose(2, 1, 3, 0, 4)))
    w2_np = _w_np(W2)
    in_maps = []
    for c in range(NCORES):
        sl = slice(RPC * c, RPC * (c + 1))
        m = {
            "xr": np.ascontiguousarray(x_flat[sl]),
            "catT": np.ascontiguousarray(catT_all[:, sl]),
            "wo": wo_np,
            "w1": w1_np,
            "w2": w2_np,
        }
        if has_bo:
            m["bo"] = bo
        if has_b1:
            m["b1"] = b1
        if not trivial:
            m["ln2w"] = ln2_w
            m["ln2b"] = ln2_b
        if has_b2:
            m["b2"] = b2
        in_maps.append(m)
    return in_maps


def run_ffn(x_flat, catT_all, Wo, bo, W1, b1, W2, b2, ln2_w, ln2_b, alpha):
    trivial = bool(np.all(ln2_w == 1.0) and np.all(ln2_b == 0.0))
    has_bo = bool(np.any(bo != 0.0))
    has_b1 = bool(np.any(b1 != 0.0))
    has_b2 = bool(np.any(b2 != 0.0))
    nc = _get_ffn_nc(not trivial, has_bo, has_b1, has_b2, alpha)
    flags = (trivial, has_bo, has_b1, has_b2)
    in_maps = ffn_in_maps(x_flat, catT_all, Wo, bo, W1, b1, W2, b2,
                          ln2_w, ln2_b, flags)
    res = run_bass_kernel_spmd(nc, in_maps, list(range(NCORES)), trace=False)
    return np.concatenate(
        [res.results[c]["out"] for c in range(NCORES)], axis=0)


def kernel(x, ln1_w, ln1_b, Wk, Wq, Wv, Wo, bo, ln2_w, ln2_b, W1, b1,
           prelu_a, W2, b2):
    x = np.asarray(x, np.float32)
    x_flat = np.ascontiguousarray(x.reshape(B * T, C))
    Wq = np.asarray(Wq, np.float32)
    Wk = np.asarray(Wk, np.float32)
    Wv = np.asarray(Wv, np.float32)
    Wo = np.asarray(Wo, np.float32)
    alpha = float(np.asarray(prelu_a))

    catT_all = run_attn(x_flat, Wq, Wk, Wv,
                        np.asarray(ln1_w, np.float32),
                        np.asarray(ln1_b, np.float32))
    out = run_ffn(x_flat, catT_all, Wo, np.asarray(bo, np.float32),
                  np.asarray(W1, np.float32), np.asarray(b1, np.float32),
                  np.asarray(W2, np.float32), np.asarray(b2, np.float32),
                  np.asarray(ln2_w, np.float32),
                  np.asarray(ln2_b, np.float32), alpha)
    return out.reshape(B, T, C).astype(np.float32)



# revision 55
# speedup vs baseline: 1.2948x; 1.0380x over previous
"""Trainium2 Bass kernel for a dense pre-LN transformer block.

B=4, T=1024, C=1024, H=16 heads (head_size 64).

Distribution over the 8 NeuronCores (two SPMD launches, host-side
reduction between them):

  Launch A (attention, head-parallel): every core runs the identical
  program on all 4 batches but with its own pair of heads (weight
  slices are per-core input data). Each core computes LN1, its 2 heads'
  q/k/v + causal attention, and the partial Wo projection of those
  heads for the whole [B*T, C] output (written f32 from PSUM).
  NOTE the reference computes scores as k @ q^T (roles of q/k swapped
  vs standard attention) — handled by using k rows as the "queries".

  Host: x2 = x + sum_c projpart_c + bo.

  Launch B (FFN, row-parallel): core c runs LN2 + W1/PReLU/W2 + residual
  on rows [512c, 512(c+1)) of x2.

Matmuls run in bf16 (1 cyc/row on the PE regardless of output width;
fp32r drops to 1/4 rate for outputs narrower than 256). LN normalizes
run on the DVE — GpSimd is ~30x slower and was the original bottleneck.
"""

from contextlib import ExitStack

import numpy as np

import concourse.bass as bass
import concourse.tile as tile
from concourse import bacc, mybir
from concourse.bass_utils import run_bass_kernel_spmd
from concourse.masks import make_identity, make_causal_mask

F32 = mybir.dt.float32
F32R = mybir.dt.float32r
BF16 = mybir.dt.bfloat16
# FFN W1/W2/fT/h2T dtype: BF16 halves the dominant 32MB weight stream
# (rel-err impact validated on HW before adoption)
FFN_WDT = BF16
AF = mybir.ActivationFunctionType
ALU = mybir.AluOpType

B, T, C, H, HS = 4, 1024, 1024, 16, 64
NCORES = 8
EPS = 1e-5
SCALE = float(C) ** -0.5  # 1/32, folded into the softmax exp
NEG = -1e30

NTB = T // 128   # 8 token blocks per batch
NCC = C // 128   # 8 channel chunks


# --------------------------------------------------------------------------
# kernel A: attention, 2 heads per core, all batches
# --------------------------------------------------------------------------

def _attn_body(ctx, tc, x, wq, wk, wv, lnw, lnb, catoutT):
    """Transposed-scores attention: scoresT[s,t] with s on partitions.

    softmax denominator comes from an appended ones-column in v (av psum
    column 64), normalization is a per-partition scale on the av output,
    so no wei transposes are needed; only [t,d]->[d,t] cat transposes.

    All matmul operands are bf16 (1 cyc/row on the PE even for <256-wide
    outputs, where fp32r drops to 1/4 rate); LN normalize runs on the DVE
    (GpSimd is ~30x slower and was the launch bottleneck).

    Each matmul pays ~75ns of weight-load overhead, so matmuls are batched
    to the widest moving dim possible: scores in <=512-wide chunks, v via
    a [d,t] projection (512-wide) + per-block PE transposes.

    The per-head outputs are written TRANSPOSED (catT [128 d, B*T]) so
    launch B can feed its Wo projection without any transposes.
    """
    nc = tc.nc

    const = ctx.enter_context(tc.tile_pool(name="const", bufs=1))
    scratch = const.tile([128, 128], F32)
    make_identity(nc, scratch)
    ident = const.tile([128, 128], BF16)
    nc.vector.tensor_copy(out=ident, in_=scratch)
    eps_t = const.tile([128, 1], F32)
    nc.vector.memset(eps_t, EPS)

    wq_sb = const.tile([128, NCC, 128], BF16, tag="wq")
    wk_sb = const.tile([128, NCC, 128], BF16, tag="wk")
    wv_sb = const.tile([128, NCC, 128], BF16, tag="wv")
    nc.sync.dma_start(out=wq_sb, in_=wq.rearrange("(cc p) d -> p cc d", p=128))
    nc.sync.dma_start(out=wk_sb, in_=wk.rearrange("(cc p) d -> p cc d", p=128))
    nc.sync.dma_start(out=wv_sb, in_=wv.rearrange("(cc p) d -> p cc d", p=128))
    general_ln = lnw is not None
    if general_ln:
        lnw_bc = const.tile([128, C], F32, tag="lnw")
        lnb_bc = const.tile([128, C], F32, tag="lnb")
        nc.sync.dma_start(
            out=lnw_bc,
            in_=bass.AP(tensor=lnw.tensor, offset=lnw.offset,
                        ap=[[0, 128]] + list(lnw.ap)))
        nc.sync.dma_start(
            out=lnb_bc,
            in_=bass.AP(tensor=lnb.tensor, offset=lnb.offset,
                        ap=[[0, 128]] + list(lnb.ap)))

    xp = ctx.enter_context(tc.tile_pool(name="xp", bufs=9))
    hp = ctx.enter_context(tc.tile_pool(name="hp", bufs=9))
    hTp = ctx.enter_context(tc.tile_pool(name="hTp", bufs=1))
    stat = ctx.enter_context(tc.tile_pool(name="stat", bufs=4))
    qkp = ctx.enter_context(tc.tile_pool(name="qkp", bufs=2))
    vp = ctx.enter_context(tc.tile_pool(name="vp", bufs=2))
    epl = ctx.enter_context(tc.tile_pool(name="epl", bufs=3))
    ctkp = ctx.enter_context(tc.tile_pool(name="ctkp", bufs=10))
    catBp = ctx.enter_context(tc.tile_pool(name="catBp", bufs=2))

    # PSUM banks: mm 2x[128,512]=2, score 2x[128,1024]=4, tr4 2x[128,512]=2
    PSM = ctx.enter_context(tc.tile_pool(name="psm", bufs=2, space="PSUM"))
    PSS = ctx.enter_context(tc.tile_pool(name="pss", bufs=2, space="PSUM"))
    PST = ctx.enter_context(tc.tile_pool(name="pst", bufs=2, space="PSUM"))

    for b in range(B):
        # ---- LN1. rstd = 1/sqrt(var+eps) via ACT Sqrt + DVE reciprocal
        # (Ln/Exp would thrash ACT tables against the softmax Exp).
        # Batch 0 is split in two groups so the first transposes don't
        # wait on all 8 tiles' stats; later batches hide under compute.
        h_tiles = []
        groups = ([range(0, 4), range(4, 8)] if b == 0
                  else [range(0, NTB)])
        for g, rng in enumerate(groups):
            n = len(rng)
            mvs = stat.tile([128, n, 2], F32, tag=f"mvs{n}",
                            name=f"mvs_{b}_{g}")
            rstd = stat.tile([128, n], F32, tag=f"rstd{n}",
                             name=f"rstd_{b}_{g}")
            sd = stat.tile([128, n], F32, tag=f"sd{n}", name=f"sd_{b}_{g}")
            xts = []
            for jj, i in enumerate(rng):
                r0 = (b * NTB + i) * 128
                xt = xp.tile([128, C], BF16, tag="x", name=f"x_{b}_{i}")
                nc.sync.dma_start(out=xt, in_=x[r0:r0 + 128, :])
                st = stat.tile([128, 2, 6], F32, tag="bn",
                               name=f"bn_{b}_{i}")
                for k in range(2):
                    nc.vector.bn_stats(out=st[:, k, :],
                                       in_=xt[:, k * 512:(k + 1) * 512])
                nc.vector.bn_aggr(out=mvs[:, jj, :], in_=st)
                xts.append(xt)
            nc.scalar.activation(out=sd, in_=mvs[:, :, 1], func=AF.Sqrt,
                                 bias=eps_t)
            nc.vector.reciprocal(out=rstd, in_=sd)
            for jj, i in enumerate(rng):
                ht = hp.tile([128, C], BF16, tag="h", name=f"h_{b}_{i}")
                nc.vector.tensor_scalar(
                    out=ht, in0=xts[jj], scalar1=mvs[:, jj, 0:1],
                    scalar2=rstd[:, jj:jj + 1], op0=ALU.subtract,
                    op1=ALU.mult)
                if general_ln:
                    nc.vector.tensor_mul(out=ht, in0=ht, in1=lnw_bc)
                    nc.vector.tensor_add(out=ht, in0=ht, in1=lnb_bc)
                h_tiles.append(ht)

        # ---- transpose h -> hT, grouped 4 blocks per psum/copy ----
        hT = hTp.tile([128, NCC, T], BF16, tag="hT")
        for cc in range(NCC):
            for g in range(2):
                pt = PST.tile([128, 512], BF16, tag="tr4",
                              name=f"pt_{b}_{cc}_{g}")
                for j in range(4):
                    i = g * 4 + j
                    nc.tensor.transpose(
                        pt[:, j * 128:(j + 1) * 128],
                        h_tiles[i][:, cc * 128:(cc + 1) * 128], ident)
                eng = nc.vector if (cc + g) % 2 else nc.scalar
                if eng is nc.scalar:
                    nc.scalar.copy(
                        out=hT[:, cc, g * 512:(g + 1) * 512], in_=pt)
                else:
                    nc.vector.tensor_copy(
                        out=hT[:, cc, g * 512:(g + 1) * 512], in_=pt)

        # ---- qkv (2 heads packed: d2 = 128) ----
        qT2 = qkp.tile([128, T], BF16, tag="qT", name=f"qT_{b}")
        kT2 = qkp.tile([128, T], BF16, tag="kT", name=f"kT_{b}")
        for tch in range(T // 512):
            tsl = slice(tch * 512, (tch + 1) * 512)
            pq = PSM.tile([128, 512], F32, tag="mm", name=f"pq_{b}_{tch}")
            for cc in range(NCC):
                nc.tensor.matmul(pq, wq_sb[:, cc, :], hT[:, cc, tsl],
                                 start=(cc == 0), stop=(cc == NCC - 1))
            nc.scalar.copy(out=qT2[:, tsl], in_=pq)
            pk = PSM.tile([128, 512], F32, tag="mm", name=f"pk_{b}_{tch}")
            for cc in range(NCC):
                nc.tensor.matmul(pk, wk_sb[:, cc, :], hT[:, cc, tsl],
                                 start=(cc == 0), stop=(cc == NCC - 1))
            nc.scalar.copy(out=kT2[:, tsl], in_=pk)
        # vT: [d2, t] via 512-wide streams (same form as q/k)
        vT2 = qkp.tile([128, T], BF16, tag="vT", name=f"vT_{b}")
        for tch in range(T // 512):
            tsl = slice(tch * 512, (tch + 1) * 512)
            pv = PSM.tile([128, 512], F32, tag="mm", name=f"pvT_{b}_{tch}")
            for cc in range(NCC):
                nc.tensor.matmul(pv, wv_sb[:, cc, :], hT[:, cc, tsl],
                                 start=(cc == 0), stop=(cc == NCC - 1))
            nc.scalar.copy(out=vT2[:, tsl], in_=pv)
        # v2: [t_part, sc, 130]: per head 65 cols (64 v + ones), built by
        # transposing vT back to [t, d] in groups of 4 blocks per psum tile
        v2 = vp.tile([128, NTB, 132], BF16, tag="v2", name=f"v2_{b}")
        nc.vector.memset(v2, 0.0)
        nc.vector.memset(v2[:, :, 64:65], 1.0)
        nc.vector.memset(v2[:, :, 130:131], 1.0)
        for g in range(2):
            ptv = PST.tile([128, 512], BF16, tag="tr4", name=f"ptv_{b}_{g}")
            for j in range(4):
                i = g * 4 + j
                nc.tensor.transpose(
                    ptv[:, j * 128:(j + 1) * 128],
                    vT2[:, i * 128:(i + 1) * 128], ident)
            for j in range(4):
                i = g * 4 + j
                nc.vector.tensor_copy(out=v2[:, i, 0:64],
                                      in_=ptv[:, j * 128:j * 128 + 64])
                nc.vector.tensor_copy(out=v2[:, i, 66:130],
                                      in_=ptv[:, j * 128 + 64:(j + 1) * 128])

        # ---- attention ----
        # Both heads' score phases run back-to-back so head h1's score
        # matmuls fill the PE while head h0's exps drain on ACT (the PE
        # is in-order: without this, each head's av stalls on its exps,
        # dropping the PE out of its fast p-state).
        cat_toks = [ctkp.tile([128, 128], BF16, tag="ctk",
                              name=f"ctk_{b}_{i}") for i in range(NTB)]
        eps_lists = [[], []]
        for h in range(2):
            hsl = slice(h * 64, (h + 1) * 64)
            for sc in range(NTB):
                W = (NTB - sc) * 128  # t columns: blocks sc..7
                pss = PSS.tile([128, W], F32, tag="score",
                               name=f"pss_{b}_{h}_{sc}")
                # one matmul per 512-aligned chunk (t cols are contiguous
                # in kT2); per-128-block matmuls waste ~75ns each on
                # weight (re)loads of the same qT block
                for n0 in range(0, W, 512):
                    n1 = min(n0 + 512, W)
                    nc.tensor.matmul(
                        pss[:, n0:n1],
                        qT2[hsl, sc * 128:(sc + 1) * 128],
                        kT2[hsl, sc * 128 + n0:sc * 128 + n1],
                        start=True, stop=True)
                e_sc = epl.tile([128, W], BF16, tag=f"e{sc}",
                                name=f"e_{b}_{h}_{sc}")
                # one exp op per PSUM bank (bank-crossing ACT reads are
                # suspect for the NRT_EXEC_UNIT_UNRECOVERABLE wedge)
                n0 = 0
                while n0 < W:
                    n1 = min(n0 + 512, W)
                    nc.scalar.activation(out=e_sc[:, n0:n1],
                                         in_=pss[:, n0:n1], func=AF.Exp,
                                         scale=SCALE)
                    n0 = n1
                # causal mask on the diagonal block: zero out s>t entries
                # post-exp (runs on the otherwise-idle gpsimd engine)
                nc.gpsimd.affine_select(
                    out=e_sc[:, 0:128], in_=e_sc[:, 0:128],
                    compare_op=ALU.is_ge, fill=0.0, base=0,
                    pattern=[[1, 128]], channel_multiplier=-1)
                eps_lists[h].append(e_sc)
        for h in range(2):
            hsl = slice(h * 64, (h + 1) * 64)
            eps_list = eps_lists[h]
            # av + normalize into cat_tok
            for i in range(NTB):
                po = PSM.tile([128, 66], F32, tag="mm",
                              name=f"po_{b}_{h}_{i}")
                for sc in range(i + 1):
                    j = i - sc
                    nc.tensor.matmul(
                        po, eps_list[sc][:, j * 128:(j + 1) * 128],
                        v2[:, sc, h * 66:(h + 1) * 66],
                        start=(sc == 0), stop=(sc == i))
                rec = stat.tile([128, 1], F32, tag="rec",
                                name=f"rec_{b}_{h}_{i}")
                nc.vector.reciprocal(out=rec, in_=po[:, 64:65])
                nc.vector.tensor_scalar_mul(
                    out=cat_toks[i][:, hsl], in0=po[:, 0:64], scalar1=rec)

        # ---- transpose cat -> [d, t] and write catT to DRAM ----
        # (launch B consumes catT directly for the Wo projection, so it
        # does no transposes and starts its matmuls immediately)
        for g in range(2):
            ptc = PST.tile([128, 512], BF16, tag="tr4", name=f"ptc_{b}_{g}")
            for j in range(4):
                nc.tensor.transpose(
                    ptc[:, j * 128:(j + 1) * 128], cat_toks[g * 4 + j], ident)
            catB = catBp.tile([128, 512], BF16, tag="catB",
                              name=f"catB_{b}_{g}")
            nc.vector.tensor_copy(out=catB, in_=ptc)
            c0 = b * T + g * 512
            nc.sync.dma_start(out=catoutT[:, c0:c0 + 512], in_=catB)


def _build_attn(general_ln: bool, repeat: int = 1):
    nc = bacc.Bacc("TRN2", target_bir_lowering=False, debug=False)
    x = nc.dram_tensor("x", [B * T, C], BF16, kind="ExternalInput").ap()
    wq = nc.dram_tensor("wq", [C, 128], BF16, kind="ExternalInput").ap()
    wk = nc.dram_tensor("wk", [C, 128], BF16, kind="ExternalInput").ap()
    wv = nc.dram_tensor("wv", [C, 128], BF16, kind="ExternalInput").ap()
    lnw = lnb = None
    if general_ln:
        lnw = nc.dram_tensor("lnw", [C], F32, kind="ExternalInput").ap()
        lnb = nc.dram_tensor("lnb", [C], F32, kind="ExternalInput").ap()
    catoutT = nc.dram_tensor("catT", [128, B * T], BF16,
                             kind="ExternalOutput").ap()
    with tile.TileContext(nc) as tc:
        for _ in range(repeat):
            with ExitStack() as ctx:
                _attn_body(ctx, tc, x, wq, wk, wv, lnw, lnb, catoutT)
    nc.compile()
    return nc


# --------------------------------------------------------------------------
# kernel B: FFN, 512 rows per core
# --------------------------------------------------------------------------

RPC = (B * T) // NCORES  # 512 rows per core
NRB = RPC // 128         # 4 row blocks
NHID = 4 * C // 128      # 32 hidden chunks


def _ffn_body(ctx, tc, xr, catT, wo, w1, w2, bo, b1, ln2w, ln2b, b2,
              alpha, out, wdt=F32R):
    """Per-core rows: proj = catT.T @ Wo (+bo); x2 = x + proj; LN2 + FFN.

    catT arrives pre-transposed from launch A, so the projection is pure
    matmul and pipelines per row-block with LN2/h2T. PReLU is a single
    scalar-engine Lrelu op. wdt: dtype for W1/W2/fT/h2T.
    """
    nc = tc.nc
    general_ln = ln2w is not None

    const = ctx.enter_context(tc.tile_pool(name="const", bufs=1))
    scratch = const.tile([128, 128], F32)
    make_identity(nc, scratch)
    ident = const.tile([128, 128], BF16)
    nc.vector.tensor_copy(out=ident, in_=scratch)
    eps_t = const.tile([128, 1], F32)
    nc.vector.memset(eps_t, EPS)
    b1_sb = None
    if b1 is not None:
        b1_sb = const.tile([128, NHID], F32, tag="b1")
        nc.sync.dma_start(out=b1_sb, in_=b1.rearrange("(h p) -> p h", p=128))

    def bcast(src, tag):
        t = const.tile([128, C], F32, tag=tag, name=tag)
        nc.sync.dma_start(
            out=t, in_=bass.AP(tensor=src.tensor, offset=src.offset,
                               ap=[[0, 128]] + list(src.ap)))
        return t

    bo_bc = bcast(bo, "bo") if bo is not None else None
    lnw_bc = bcast(ln2w, "lnw") if general_ln else None
    lnb_bc = bcast(ln2b, "lnb") if general_ln else None
    b2_bc = bcast(b2, "b2") if b2 is not None else None

    # per-cc DMA chunks so the first proj matmul only waits on chunk 0
    wo_sb = const.tile([128, NCC, C], BF16, tag="wo")
    wo_r = wo.rearrange("(cc p) c -> p cc c", p=128)
    ctT = const.tile([128, NCC, RPC], BF16, tag="catT")
    ctT_r = catT.rearrange("(cc p) t -> p cc t", p=128)
    for cc in range(NCC):
        nc.sync.dma_start(out=ctT[:, cc, :], in_=ctT_r[:, cc, :])
        nc.sync.dma_start(out=wo_sb[:, cc, :], in_=wo_r[:, cc, :])

    xrp = ctx.enter_context(tc.tile_pool(name="xrp", bufs=2))
    x2p = ctx.enter_context(tc.tile_pool(name="x2p", bufs=NRB))
    hp = ctx.enter_context(tc.tile_pool(name="hp", bufs=2))
    h2Tp = ctx.enter_context(tc.tile_pool(name="h2Tp", bufs=1))
    stat = ctx.enter_context(tc.tile_pool(name="stat", bufs=8))
    w1p = ctx.enter_context(tc.tile_pool(name="w1p", bufs=4))
    w2p = ctx.enter_context(tc.tile_pool(name="w2p", bufs=4))
    ftp = ctx.enter_context(tc.tile_pool(name="ftp", bufs=NHID))
    tmp = ctx.enter_context(tc.tile_pool(name="tmp", bufs=3))
    osb = ctx.enter_context(tc.tile_pool(name="osb", bufs=2))

    x2_tiles = []
    h2T = h2Tp.tile([128, NCC, RPC], wdt, tag="h2T")
    with tc.tile_pool(name="psp", bufs=2, space="PSUM") as PSP, \
         tc.tile_pool(name="pst", bufs=2, space="PSUM") as PST:
        # ---- proj + residual + LN2 + transpose, pipelined per r ----
        for r in range(NRB):
            rsl = slice(r * 128, (r + 1) * 128)
            xt = xrp.tile([128, C], F32, tag="xr", name=f"xr_{r}")
            nc.sync.dma_start(out=xt, in_=xr[rsl, :])
            pps = PSP.tile([128, C], F32, tag="pp", name=f"pp_{r}")
            for cc in range(NCC):
                for co in range(2):
                    csl = slice(co * 512, (co + 1) * 512)
                    nc.tensor.matmul(pps[:, csl], ctT[:, cc, rsl],
                                     wo_sb[:, cc, csl],
                                     start=(cc == 0), stop=(cc == NCC - 1))
            x2t = x2p.tile([128, C], F32, tag="x2", name=f"x2_{r}")
            nc.vector.tensor_add(out=x2t, in0=pps, in1=xt)
            if bo_bc is not None:
                nc.vector.tensor_add(out=x2t, in0=x2t, in1=bo_bc)
            x2_tiles.append(x2t)
            # LN2 on this row block
            st = stat.tile([128, 2, 6], F32, tag="bn", name=f"bn_{r}")
            for k in range(2):
                nc.vector.bn_stats(out=st[:, k, :],
                                   in_=x2t[:, k * 512:(k + 1) * 512])
            mv = stat.tile([128, 2], F32, tag="mv", name=f"mv_{r}")
            nc.vector.bn_aggr(out=mv, in_=st)
            sd = stat.tile([128, 1], F32, tag="sd", name=f"sd_{r}")
            nc.scalar.activation(out=sd, in_=mv[:, 1:2], func=AF.Sqrt,
                                 bias=eps_t)
            rstd = stat.tile([128, 1], F32, tag="rstd", name=f"rstd_{r}")
            nc.vector.reciprocal(out=rstd, in_=sd)
            ht = hp.tile([128, C], BF16, tag="h", name=f"h_{r}")
            nc.vector.tensor_scalar(
                out=ht, in0=x2t, scalar1=mv[:, 0:1], scalar2=rstd,
                op0=ALU.subtract, op1=ALU.mult)
            if general_ln:
                nc.vector.tensor_mul(out=ht, in0=ht, in1=lnw_bc)
                nc.vector.tensor_add(out=ht, in0=ht, in1=lnb_bc)
            for g in range(2):
                pt = PST.tile([128, 4, 128], BF16, tag="tr4",
                              name=f"pt_{r}_{g}")
                for j in range(4):
                    cc = g * 4 + j
                    nc.tensor.transpose(pt[:, j, :],
                                        ht[:, cc * 128:(cc + 1) * 128], ident)
                nc.scalar.copy(out=h2T[:, g * 4:(g + 1) * 4, rsl], in_=pt)

    with tc.tile_pool(name="psf", bufs=2, space="PSUM") as PSF:
        # ---- phase 1: fT[h] = prelu(W1_h^T @ h2 + b1) ----
        # w1 arrives host-transposed as [NHID/2, 128, 2, NCC, 128] so each
        # DMA reads contiguous 4KB partition lines (2 h-chunks per DMA)
        f_tiles = []
        for h2i in range(NHID // 2):
            w1_sb = w1p.tile([128, 2, NCC, 128], wdt, tag="w1",
                             name=f"w1_{h2i}")
            nc.sync.dma_start(out=w1_sb, in_=w1[h2i])
            for s in range(2):
                h = 2 * h2i + s
                pf = PSF.tile([128, RPC], F32, tag="ft", name=f"pf_{h}")
                for cc in range(NCC):
                    nc.tensor.matmul(pf, w1_sb[:, s, cc, :], h2T[:, cc, :],
                                     start=(cc == 0), stop=(cc == NCC - 1))
                ft = ftp.tile([128, RPC], wdt, tag="ft", name=f"ft_{h}")
                if b1_sb is not None:
                    src = tmp.tile([128, RPC], F32, tag="pb", name=f"pb_{h}")
                    nc.vector.tensor_scalar_add(out=src, in0=pf,
                                                scalar1=b1_sb[:, h:h + 1])
                else:
                    src = pf
                # PReLU: ft = src + (alpha-1)*min(src, 0). (AF.Lrelu's
                # alpha operand is ignored by this lowering — plain relu.)
                tneg = tmp.tile([128, RPC], F32, tag="tneg", name=f"tneg_{h}")
                nc.vector.tensor_scalar(
                    out=tneg, in0=src, scalar1=0.0,
                    scalar2=alpha - 1.0, op0=ALU.min, op1=ALU.mult)
                nc.vector.tensor_add(out=ft, in0=src, in1=tneg)
                f_tiles.append(ft)

    # ---- phase 2: out = fT.T @ W2 (+b2) + x2 ----
    with tc.tile_pool(name="pso", bufs=NRB, space="PSUM") as PSO:
        pouts = [PSO.tile([128, C], F32, tag="out", name=f"pout{r}")
                 for r in range(NRB)]
        for h2i in range(NHID // 2):
            w2_sb = w2p.tile([128, 2, C], wdt, tag="w2", name=f"w2_{h2i}")
            nc.sync.dma_start(
                out=w2_sb,
                in_=w2[h2i * 256:(h2i + 1) * 256, :].rearrange(
                    "(s p) c -> p s c", p=128))
            for s in range(2):
                h = 2 * h2i + s
                for r in range(NRB):
                    for co in range(2):
                        csl = slice(co * 512, (co + 1) * 512)
                        nc.tensor.matmul(pouts[r][:, csl],
                                         f_tiles[h][:, r * 128:(r + 1) * 128],
                                         w2_sb[:, s, csl],
                                         start=(h == 0), stop=(h == NHID - 1))
        for r in range(NRB):
            o_sb = osb.tile([128, C], F32, tag="o", name=f"o_{r}")
            nc.vector.tensor_add(out=o_sb, in0=pouts[r], in1=x2_tiles[r])
            if b2_bc is not None:
                nc.vector.tensor_add(out=o_sb, in0=o_sb, in1=b2_bc)
            nc.sync.dma_start(out=out[r * 128:(r + 1) * 128, :], in_=o_sb)


def _build_ffn(general_ln: bool, has_bo: bool, has_b1: bool, has_b2: bool,
               alpha: float, repeat: int = 1, wdt=F32R):
    nc = bacc.Bacc("TRN2", target_bir_lowering=False, debug=False)
    xr = nc.dram_tensor("xr", [RPC, C], F32, kind="ExternalInput").ap()
    catT = nc.dram_tensor("catT", [C, RPC], BF16, kind="ExternalInput").ap()
    wo = nc.dram_tensor("wo", [C, C], BF16, kind="ExternalInput").ap()
    # w1 is host-pretransposed to [h-pair, partition, s, cc, q] so each
    # per-pair DMA reads one contiguous 4KB line per partition
    w1 = nc.dram_tensor("w1", [NHID // 2, 128, 2, NCC, 128], wdt,
                        kind="ExternalInput").ap()
    w2 = nc.dram_tensor("w2", [4 * C, C], wdt, kind="ExternalInput").ap()
    bo = b1 = ln2w = ln2b = b2 = None
    if has_bo:
        bo = nc.dram_tensor("bo", [C], F32, kind="ExternalInput").ap()
    if has_b1:
        b1 = nc.dram_tensor("b1", [4 * C], F32, kind="ExternalInput").ap()
    if general_ln:
        ln2w = nc.dram_tensor("ln2w", [C], F32, kind="ExternalInput").ap()
        ln2b = nc.dram_tensor("ln2b", [C], F32, kind="ExternalInput").ap()
    if has_b2:
        b2 = nc.dram_tensor("b2", [C], F32, kind="ExternalInput").ap()
    out = nc.dram_tensor("out", [RPC, C], F32, kind="ExternalOutput").ap()
    with tile.TileContext(nc) as tc:
        for _ in range(repeat):
            with ExitStack() as ctx:
                _ffn_body(ctx, tc, xr, catT, wo, w1, w2, bo, b1,
                          ln2w, ln2b, b2, alpha, out, wdt=wdt)
    nc.compile()
    return nc


# --------------------------------------------------------------------------
# host orchestration
# --------------------------------------------------------------------------

_NC_CACHE = {}


def _get_attn_nc(general_ln):
    key = ("attn", general_ln)
    if key not in _NC_CACHE:
        _NC_CACHE[key] = _build_attn(general_ln)
    return _NC_CACHE[key]


def _get_ffn_nc(general_ln, has_bo, has_b1, has_b2, alpha, wdt=None):
    wdt = FFN_WDT if wdt is None else wdt
    key = ("ffn", general_ln, has_bo, has_b1, has_b2, float(alpha), wdt)
    if key not in _NC_CACHE:
        _NC_CACHE[key] = _build_ffn(general_ln, has_bo, has_b1, has_b2,
                                    float(alpha), wdt=wdt)
    return _NC_CACHE[key]


def _w_np(a):
    if FFN_WDT == BF16:
        import ml_dtypes
        return np.ascontiguousarray(a.astype(ml_dtypes.bfloat16))
    return a


def _bf(a):
    import ml_dtypes
    return np.ascontiguousarray(np.asarray(a).astype(ml_dtypes.bfloat16))


def attn_in_maps(x_flat, Wq, Wk, Wv, trivial, ln1_w, ln1_b):
    x_bf = _bf(x_flat)
    in_maps = []
    for c in range(NCORES):
        h0 = 2 * c
        m = {
            "x": x_bf,
            "wq": _bf(np.concatenate([Wq[h0], Wq[h0 + 1]], axis=1)),
            "wk": _bf(np.concatenate([Wk[h0], Wk[h0 + 1]], axis=1)),
            "wv": _bf(np.concatenate([Wv[h0], Wv[h0 + 1]], axis=1)),
        }
        if not trivial:
            m["lnw"] = ln1_w
            m["lnb"] = ln1_b
        in_maps.append(m)
    return in_maps


def run_attn(x_flat, Wq, Wk, Wv, ln1_w, ln1_b):
    """Returns catT [C, B*T] bf16: transposed per-head attention outputs."""
    trivial = bool(np.all(ln1_w == 1.0) and np.all(ln1_b == 0.0))
    nc = _get_attn_nc(not trivial)
    in_maps = attn_in_maps(x_flat, Wq, Wk, Wv, trivial, ln1_w, ln1_b)
    res = run_bass_kernel_spmd(nc, in_maps, list(range(NCORES)), trace=False)
    return np.concatenate(
        [res.results[c]["catT"] for c in range(NCORES)], axis=0)


def ffn_in_maps(x_flat, catT_all, Wo, bo, W1, b1, W2, b2, ln2_w, ln2_b,
                flags):
    trivial, has_bo, has_b1, has_b2 = flags
    wo_np = _bf(Wo)
    # pre-transpose W1 to [h-pair, p, s, cc, q] (see _build_ffn)
    w1_np = _w_np(np.ascontiguousarray(
        W1.reshape(NCC, 128, NHID // 2, 2, 128).transpose(2, 1, 3, 0, 4)))
    w2_np = _w_np(W2)
    in_maps = []
    for c in range(NCORES):
        sl = slice(RPC * c, RPC * (c + 1))
        m = {
            "xr": np.ascontiguousarray(x_flat[sl]),
            "catT": np.ascontiguousarray(catT_all[:, sl]),
            "wo": wo_np,
            "w1": w1_np,
            "w2": w2_np,
        }
        if has_bo:
            m["bo"] = bo
        if has_b1:
            m["b1"] = b1
        if not trivial:
            m["ln2w"] = ln2_w
            m["ln2b"] = ln2_b
        if has_b2:
            m["b2"] = b2
        in_maps.append(m)
    return in_maps


def run_ffn(x_flat, catT_all, Wo, bo, W1, b1, W2, b2, ln2_w, ln2_b, alpha):
    trivial = bool(np.all(ln2_w == 1.0) and np.all(ln2_b == 0.0))
    has_bo = bool(np.any(bo != 0.0))
    has_b1 = bool(np.any(b1 != 0.0))
    has_b2 = bool(np.any(b2 != 0.0))
    nc = _get_ffn_nc(not trivial, has_bo, has_b1, has_b2, alpha)
    flags = (trivial, has_bo, has_b1, has_b2)
    in_maps = ffn_in_maps(x_flat, catT_all, Wo, bo, W1, b1, W2, b2,
                          ln2_w, ln2_b, flags)
    res = run_bass_kernel_spmd(nc, in_maps, list(range(NCORES)), trace=False)
    return np.concatenate(
        [res.results[c]["out"] for c in range(NCORES)], axis=0)


def kernel(x, ln1_w, ln1_b, Wk, Wq, Wv, Wo, bo, ln2_w, ln2_b, W1, b1,
           prelu_a, W2, b2):
    x = np.asarray(x, np.float32)
    x_flat = np.ascontiguousarray(x.reshape(B * T, C))
    Wq = np.asarray(Wq, np.float32)
    Wk = np.asarray(Wk, np.float32)
    Wv = np.asarray(Wv, np.float32)
    Wo = np.asarray(Wo, np.float32)
    alpha = float(np.asarray(prelu_a))

    catT_all = run_attn(x_flat, Wq, Wk, Wv,
                        np.asarray(ln1_w, np.float32),
                        np.asarray(ln1_b, np.float32))
    out = run_ffn(x_flat, catT_all, Wo, np.asarray(bo, np.float32),
                  np.asarray(W1, np.float32), np.asarray(b1, np.float32),
                  np.asarray(W2, np.float32), np.asarray(b2, np.float32),
                  np.asarray(ln2_w, np.float32),
                  np.asarray(ln2_b, np.float32), alpha)
    return out.reshape(B, T, C).astype(np.float32)

